# revision 7
# baseline (speedup 1.0000x reference)
"""Trainium2 Bass kernel: NeuralGrangerCausality (GCN + causal attention + GRU).

Strategy (8 NeuronCores, SPMD):
  - Phase A (T-sharded, 8 timesteps/core): lin_in -> causal softmax-matmul
    -> fusion -> GCN x2 (dense normalized-adjacency matmul, built on host)
    all as bf16 PE matmuls with fp32 PSUM accumulation.
    Per-timestep node tensors are kept feature-major [H, N]; aggregation
    matmuls contract over nodes with 2-timestep-stacked lhsT ([j, 2t*h]) for
    full 128-wide PE utilization.
  - Reshard T->N via AllToAll (bf16, 2MB).
  - Phase B (N-sharded, 250 nodes/core): 2-layer GRU recurrence (input-gate
    matmuls batched, per-step Whh matmuls + gate math), BN+ReLU, lin_out,
    log_softmax.

kernel(**inputs) takes the FULL inputs, preps/shards on host (numpy only:
index->dense adjacency, transposes, BN folding), runs the NEFF on cores 0-7
via run_bass_kernel_spmd, and concatenates the per-core [250, 10] outputs.
"""

import os
import sys

import numpy as np

for _p in ("/opt/trn_rl_repo", "/root/.axon_site/_ro/trn_rl_repo"):
    if os.path.isdir(_p) and _p not in sys.path:
        sys.path.append(_p)

import ml_dtypes  # noqa: E402

import concourse.bass as bass  # noqa: E402,F401
import concourse.mybir as mybir  # noqa: E402
import concourse.tile as tile  # noqa: E402
from concourse import bacc  # noqa: E402
from concourse.bass_utils import run_bass_kernel_spmd  # noqa: E402
from concourse.masks import make_identity  # noqa: E402

AF = mybir.ActivationFunctionType
F32 = mybir.dt.float32
BF16 = mybir.dt.bfloat16
BF = ml_dtypes.bfloat16

T, N, F_IN, H, C = 64, 2000, 32, 64, 10
W = 8            # cores
TL = T // W      # timesteps per core (phase A)
NL = N // W      # nodes per core (phase B)
EPS = 1e-5
NJ = (N + 127) // 128                       # node contraction chunks
JCH = [(j * 128, min(128, N - j * 128)) for j in range(NJ)]
NI = 4
IC = N // NI                                # 500-wide free-dim chunks
ICH = [(i * IC, IC) for i in range(NI)]

_CACHE: dict = {}


# --------------------------------------------------------------------------
# host-side prep
# --------------------------------------------------------------------------

def _prep(inputs):
    f32 = np.float32
    g = {k: np.asarray(v) for k, v in inputs.items()}

    x_seq = g["x_seq"].astype(f32)                       # [T, N, F]
    src = g["edge_index"][0].astype(np.int64)
    dst = g["edge_index"][1].astype(np.int64)
    ew = g["edge_weight"].astype(f32)

    # GCN normalization with self loops (PyG gcn_norm, fill 1)
    loops = np.arange(N, dtype=np.int64)
    src_f = np.concatenate([src, loops])
    dst_f = np.concatenate([dst, loops])
    w_f = np.concatenate([ew, np.ones(N, f32)])
    deg = np.zeros(N, f32)
    np.add.at(deg, dst_f, w_f)
    dis = (1.0 / np.sqrt(np.maximum(deg, 1e-12))).astype(f32)
    norm = dis[src_f] * w_f * dis[dst_f]
    at = np.zeros((N, N), f32)                           # at[j, i] = A[i, j]
    np.add.at(at, (src_f, dst_f), norm)

    # x_seq feature-major with an appended ones-row (bias via matmul aug)
    xs = np.concatenate(
        [x_seq.transpose(0, 2, 1), np.ones((T, 1, N), f32)], axis=1
    )                                                    # [T, F+1, N]

    cwt = np.ascontiguousarray(g["causal_weight"].astype(f32).T)  # [j, i]

    linw = np.concatenate(
        [g["lin_in_w"].astype(f32).T, g["lin_in_b"].astype(f32)[None]], axis=0
    )                                                    # [F+1, H]

    fw = g["fusion_w"].astype(f32)                       # [H, 2H]
    fusw1d = np.concatenate([fw[:, :H].T, fw[:, :H].T], 0)   # [2H, H]
    fusw2d = np.concatenate([fw[:, H:].T, fw[:, H:].T], 0)
    fusbd = np.tile(g["fusion_b"].astype(f32), 2)[:, None]   # [2H, 1]

    def gcn_fold(wk, bk, gk, bbk, mk, vk):
        sc = g[gk].astype(f32) / np.sqrt(g[vk].astype(f32) + EPS)
        wt = g[wk].astype(f32).T * sc[None, :]           # [H_in, H_out]
        bias = (g[bk].astype(f32) - g[mk].astype(f32)) * sc + g[bbk].astype(f32)
        return (np.concatenate([wt, wt], 0),
                np.tile(bias, 2)[:, None])

    w0d, b0d = gcn_fold("gcn_w0", "gcn_b0", "bn0_g", "bn0_b", "bn0_m", "bn0_v")
    w1d, b1d = gcn_fold("gcn_w1", "gcn_b1", "bn1_g", "bn1_b", "bn1_m", "bn1_v")

    p = {
        "cwt": cwt, "at": at.astype(BF),
        "linw": linw.astype(BF),
        "fusw1d": fusw1d.astype(BF), "fusw2d": fusw2d.astype(BF),
        "fusbd": fusbd,
        "w0d": w0d.astype(BF), "b0d": b0d,
        "w1d": w1d.astype(BF), "b1d": b1d,
    }
    for layer in (0, 1):
        wih = g[f"gru_wih{layer}"].astype(f32)            # [3H, H]
        whh = g[f"gru_whh{layer}"].astype(f32)
        bih = g[f"gru_bih{layer}"].astype(f32)
        bhh = g[f"gru_bhh{layer}"].astype(f32)
        p[f"wih{layer}"] = wih.T.astype(BF)               # [H, 3H]
        p[f"whh{layer}"] = whh.T.astype(BF)
        p[f"girzb{layer}"] = (bih + bhh)[: 2 * H, None].astype(f32)
        p[f"ginb{layer}"] = bih[2 * H:, None].astype(f32)
        p[f"bhnb{layer}"] = bhh[2 * H:, None].astype(f32)

    scout = g["bnout_g"].astype(f32) / np.sqrt(g["bnout_v"].astype(f32) + EPS)
    p["scout"] = scout[:, None]
    p["bout"] = (g["bnout_b"].astype(f32)
                 - g["bnout_m"].astype(f32) * scout)[:, None]
    p["loutw"] = np.concatenate(
        [g["lin_out_w"].astype(f32).T, g["lin_out_b"].astype(f32)[None]], 0
    ).astype(BF)                                          # [H+1, C]

    xs_bf = xs.astype(BF)
    in_maps = []
    for c in range(W):
        m = dict(p)
        m["xs"] = np.ascontiguousarray(xs_bf[c * TL:(c + 1) * TL])
        in_maps.append(m)
    return in_maps


# --------------------------------------------------------------------------
# kernel IR
# --------------------------------------------------------------------------

def _emit(nc, tc):
    def param(name, shape, dt):
        return nc.dram_tensor(name, shape, dt, kind="ExternalInput").ap()

    xs = param("xs", [TL, F_IN + 1, N], BF16)
    cwt = param("cwt", [N, N], F32)
    at = param("at", [N, N], BF16)
    linw = param("linw", [F_IN + 1, H], BF16)
    fusw1d = param("fusw1d", [2 * H, H], BF16)
    fusw2d = param("fusw2d", [2 * H, H], BF16)
    fusbd = param("fusbd", [2 * H, 1], F32)
    w0d = param("w0d", [2 * H, H], BF16)
    b0d = param("b0d", [2 * H, 1], F32)
    w1d = param("w1d", [2 * H, H], BF16)
    b1d = param("b1d", [2 * H, 1], F32)
    wih = [param(f"wih{l}", [H, 3 * H], BF16) for l in (0, 1)]
    whh = [param(f"whh{l}", [H, 3 * H], BF16) for l in (0, 1)]
    girzb = [param(f"girzb{l}", [2 * H, 1], F32) for l in (0, 1)]
    ginb = [param(f"ginb{l}", [H, 1], F32) for l in (0, 1)]
    bhnb = [param(f"bhnb{l}", [H, 1], F32) for l in (0, 1)]
    scout = param("scout", [H, 1], F32)
    bout = param("bout", [H, 1], F32)
    loutw = param("loutw", [H + 1, C], BF16)
    out = nc.dram_tensor("out", [NL, C], F32, kind="ExternalOutput").ap()

    with tc.tile_pool(name="consts", bufs=1) as cst, \
         tc.tile_pool(name="dram", bufs=1, space="DRAM") as dram, \
         tc.tile_pool(name="ps", bufs=8, space="PSUM") as ps:

        def psum(pn, pshape):
            return ps.tile(pshape, F32, tag="ps", name=pn,
                           padded_shape=[128, 512])

        # ---- constants into SBUF
        def cload(ap_, cn):
            t_ = cst.tile(list(ap_.shape), ap_.dtype, name=cn, tag=cn)
            nc.sync.dma_start(out=t_, in_=ap_)
            return t_

        s_linw = cload(linw, "s_linw")
        s_fusw1 = cload(fusw1d, "s_fusw1")
        s_fusw2 = cload(fusw2d, "s_fusw2")
        s_fusb = cload(fusbd, "s_fusb")
        s_w0 = cload(w0d, "s_w0")
        s_b0 = cload(b0d, "s_b0")
        s_w1 = cload(w1d, "s_w1")
        s_b1 = cload(b1d, "s_b1")
        s_wih = [cload(wih[l], f"s_wih{l}") for l in (0, 1)]
        s_whh = [cload(whh[l], f"s_whh{l}") for l in (0, 1)]
        s_girzb = [cload(girzb[l], f"s_girzb{l}") for l in (0, 1)]
        s_ginb = [cload(ginb[l], f"s_ginb{l}") for l in (0, 1)]
        s_bhnb = [cload(bhnb[l], f"s_bhnb{l}") for l in (0, 1)]
        s_scout = cload(scout, "s_scout")
        s_bout = cload(bout, "s_bout")
        s_loutw = cload(loutw, "s_loutw")

        ones_col = cst.tile([128, 1], BF16, name="ones_col", tag="ones_col")
        nc.vector.memset(ones_col, 1.0)
        id_bf = cst.tile([128, 128], BF16, name="id_bf", tag="id_bf")
        make_identity(nc, id_bf)
        id_f32 = cst.tile([16, 16], F32, name="id_f32", tag="id_f32")
        make_identity(nc, id_f32)

        a2a_in = dram.tile([W, TL, H, NL], BF16, name="a2a_in", tag="a2a_in")
        a2a_out = dram.tile([W, TL, H, NL], BF16, name="a2a_out",
                            tag="a2a_out")

        # ================= PHASE A (T-sharded) =================
        with tc.tile_pool(name="pa", bufs=1) as pa, \
             tc.tile_pool(name="st", bufs=3) as st:

            # big per-t-pair stacked tiles
            x1nm = [pa.tile([128, NJ, 128], BF16, name=f"x1nm{q}",
                            tag=f"x1nm{q}") for q in range(TL // 2)]
            x1p = [pa.tile([128, N], BF16, name=f"x1p{q}", tag=f"x1p{q}")
                   for q in range(TL // 2)]
            xap = [pa.tile([128, N], BF16, name=f"xap{q}", tag=f"xap{q}")
                   for q in range(TL // 2)]
            x2p = [pa.tile([128, N], BF16, name=f"x2p{q}", tag=f"x2p{q}")
                   for q in range(TL // 2)]
            x3p = [pa.tile([128, N], BF16, name=f"x3p{q}", tag=f"x3p{q}")
                   for q in range(TL // 2)]
            x4p = [pa.tile([128, N], BF16, name=f"x4p{q}", tag=f"x4p{q}")
                   for q in range(TL // 2)]

            # ---- stage 1: x1 = relu(lin_in(x)) in both layouts
            for t in range(TL):
                q, o = t // 2, t % 2
                hs = slice(64 * o, 64 * o + 64)
                xst = st.tile([F_IN + 1, N], BF16, name=f"xs{t}", tag="xs")
                nc.sync.dma_start(out=xst, in_=xs[t])
                # feature-major into x1p halves
                for i, (i0, iw) in enumerate(ICH):
                    pfm = psum(f"pfm{t}_{i}", [128, IC])
                    nc.tensor.matmul(pfm[hs, :], lhsT=s_linw,
                                     rhs=xst[:, i0:i0 + iw],
                                     start=True, stop=True)
                    nc.scalar.activation(x1p[q][hs, i0:i0 + iw], pfm[hs, :],
                                         AF.Relu)
                # node-major into x1nm column halves
                for j, (j0, pj) in enumerate(JCH):
                    pnm = psum(f"pnm{t}_{j}", [128, H])
                    nc.tensor.matmul(pnm[0:pj, :], lhsT=xst[:, j0:j0 + pj],
                                     rhs=s_linw, start=True, stop=True)
                    nc.scalar.activation(
                        x1nm[q][0:pj, j, 64 * o:64 * o + 64],
                        pnm[0:pj, :], AF.Relu)

            # ---- stage 2: x_agg = softmax(cw) @ x1  (+ column rescale)
            for i, (i0, iw) in enumerate(ICH):
                pcs = psum(f"pcs{i}", [1, IC])
                pagg = [psum(f"pagg{i}_{q}", [128, IC])
                        for q in range(TL // 2)]
                for j, (j0, pj) in enumerate(JCH):
                    cwf = st.tile([128, IC], F32, name=f"cwf{i}_{j}",
                                  tag="cwf")
                    nc.sync.dma_start(out=cwf[0:pj, :],
                                      in_=cwt[j0:j0 + pj, i0:i0 + iw])
                    wct = st.tile([128, IC], BF16, name=f"wct{i}_{j}",
                                  tag="wct")
                    nc.scalar.activation(wct[0:pj, :], cwf[0:pj, :], AF.Exp)
                    nc.tensor.matmul(pcs, lhsT=ones_col[0:pj, :],
                                     rhs=wct[0:pj, :],
                                     start=(j == 0), stop=(j == NJ - 1))
                    for q in range(TL // 2):
                        nc.tensor.matmul(pagg[q],
                                         lhsT=x1nm[q][0:pj, j, :],
                                         rhs=wct[0:pj, :],
                                         start=(j == 0), stop=(j == NJ - 1))
                rinv = st.tile([1, IC], F32, name=f"rinv{i}", tag="rinv",
                               bufs=2)
                nc.vector.reciprocal(rinv, pcs)
                rb = st.tile([128, IC], F32, name=f"rb{i}", tag="rb", bufs=2)
                nc.gpsimd.partition_broadcast(rb, rinv)
                for q in range(TL // 2):
                    for o in (0, 1):
                        hs = slice(64 * o, 64 * o + 64)
                        nc.vector.tensor_mul(xap[q][hs, i0:i0 + iw],
                                             pagg[q][hs, :], rb[hs, :])

            # ---- stage 3: fusion x2 = relu(W1@x1 + W2@xagg + b)
            for q in range(TL // 2):
                for i, (i0, iw) in enumerate(ICH):
                    pf = psum(f"pf{q}_{i}", [128, IC])
                    for o in (0, 1):
                        hs = slice(64 * o, 64 * o + 64)
                        nc.tensor.matmul(pf[hs, :], lhsT=s_fusw1[hs, :],
                                         rhs=x1p[q][hs, i0:i0 + iw],
                                         start=True, stop=False)
                        nc.tensor.matmul(pf[hs, :], lhsT=s_fusw2[hs, :],
                                         rhs=xap[q][hs, i0:i0 + iw],
                                         start=False, stop=True)
                        nc.scalar.activation(x2p[q][hs, i0:i0 + iw],
                                             pf[hs, :], AF.Relu,
                                             bias=s_fusb[hs, :])

            # ---- GCN layer: z = (x @ Wsc) node-major, agg = A @ z, BN+ReLU
            def gcn_layer(xin, w_dup, b_dup, xout, evac):
                znm = [pa.tile([128, NJ, 128], BF16, name=f"znm{q}",
                               tag=f"znm{q}", bufs=2)
                       for q in range(TL // 2)]
                for t in range(TL):
                    q, o = t // 2, t % 2
                    hs = slice(64 * o, 64 * o + 64)
                    for j, (j0, pj) in enumerate(JCH):
                        pz = psum(f"pz{t}_{j}", [128, H])
                        nc.tensor.matmul(pz[0:pj, :],
                                         lhsT=xin[q][hs, j0:j0 + pj],
                                         rhs=w_dup[hs, :],
                                         start=True, stop=True)
                        nc.vector.tensor_copy(
                            znm[q][0:pj, j, 64 * o:64 * o + 64], pz[0:pj, :])
                for i, (i0, iw) in enumerate(ICH):
                    pagg = [psum(f"pag{i}_{q}", [128, IC])
                            for q in range(TL // 2)]
                    for j, (j0, pj) in enumerate(JCH):
                        atb = st.tile([128, IC], BF16, name=f"atb{i}_{j}",
                                      tag="atb", bufs=4)
                        nc.sync.dma_start(out=atb[0:pj, :],
                                          in_=at[j0:j0 + pj, i0:i0 + iw])
                        for q in range(TL // 2):
                            nc.tensor.matmul(pagg[q],
                                             lhsT=znm[q][0:pj, j, :],
                                             rhs=atb[0:pj, :],
                                             start=(j == 0),
                                             stop=(j == NJ - 1))
                    for q in range(TL // 2):
                        for o in (0, 1):
                            hs = slice(64 * o, 64 * o + 64)
                            evac(pagg[q], q, o, hs, i0, iw, b_dup, xout)

            def evac_bn(pagg, q, o, hs, i0, iw, b_dup, xout):
                nc.scalar.activation(xout[q][hs, i0:i0 + iw], pagg[hs, :],
                                     AF.Relu, bias=b_dup[hs, :])

            gcn_layer(x2p, s_w0, s_b0, x3p, evac_bn)
            gcn_layer(x3p, s_w1, s_b1, x4p, evac_bn)

            # ---- ship x4 to the all-to-all buffer
            for t in range(TL):
                q, o = t // 2, t % 2
                hs = slice(64 * o, 64 * o + 64)
                for d in range(W):
                    nc.sync.dma_start(
                        out=a2a_in[d, t],
                        in_=x4p[q][hs, d * NL:(d + 1) * NL])

        # ================= reshard T -> N =================
        nc.gpsimd.collective_compute(
            "AllToAll", mybir.AluOpType.bypass,
            replica_groups=[list(range(W))],
            ins=[a2a_in.opt()], outs=[a2a_out.opt()])

        # ================= PHASE B (N-sharded GRU) =================
        with tc.tile_pool(name="pb", bufs=1) as pb, \
             tc.tile_pool(name="gs", bufs=4) as gs:

            x4all = pb.tile([H, W, TL, NL], BF16, name="x4all", tag="x4all")
            nc.sync.dma_start(out=x4all,
                              in_=a2a_out.rearrange("s t h n -> h s t n"))
            x4f = x4all.rearrange("h s t n -> h (s t n)")

            # layer-0 input gates, batched over pairs of timesteps
            gi0rz = pb.tile([2 * H, T, NL], BF16, name="gi0rz", tag="gi0rz")
            gi0n = pb.tile([H, T, NL], BF16, name="gi0n", tag="gi0n")
            gi0rzf = gi0rz.rearrange("p t n -> p (t n)")
            gi0nf = gi0n.rearrange("p t n -> p (t n)")
            for p2 in range(T // 2):
                csl = slice(2 * p2 * NL, (2 * p2 + 2) * NL)
                rhs = x4f[:, csl]
                prz = psum(f"prz{p2}", [128, 2 * NL])
                nc.tensor.matmul(prz, lhsT=s_wih[0][:, 0:128], rhs=rhs,
                                 start=True, stop=True)
                nc.scalar.activation(gi0rzf[:, csl], prz,
                                     AF.Identity, bias=s_girzb[0])
                pn = psum(f"pn{p2}", [H, 2 * NL])
                nc.tensor.matmul(pn, lhsT=s_wih[0][:, 128:192], rhs=rhs,
                                 start=True, stop=True)
                nc.scalar.activation(gi0nf[:, csl], pn,
                                     AF.Identity, bias=s_ginb[0])

            def gru_step(layer, t, h_prev, grz, gn):
                prz = psum(f"przs{layer}_{t}", [128, NL])
                nc.tensor.matmul(prz, lhsT=id_bf, rhs=grz,
                                 start=True, stop=False)
                nc.tensor.matmul(prz, lhsT=s_whh[layer][:, 0:128],
                                 rhs=h_prev, start=False, stop=True)
                pnn = psum(f"pnn{layer}_{t}", [H, NL])
                nc.tensor.matmul(pnn, lhsT=s_whh[layer][:, 128:192],
                                 rhs=h_prev, start=True, stop=True)
                r_t = gs.tile([H, NL], BF16, name=f"r{layer}_{t}", tag="r")
                nc.scalar.activation(r_t, prz[0:64, :], AF.Sigmoid)
                z_t = gs.tile([H, NL], BF16, name=f"z{layer}_{t}", tag="z")
                nc.scalar.activation(z_t, prz[64:128, :], AF.Sigmoid)
                u_t = gs.tile([H, NL], BF16, name=f"u{layer}_{t}", tag="u")
                nc.scalar.activation(u_t, pnn, AF.Identity, bias=s_bhnb[layer])
                v_t = gs.tile([H, NL], BF16, name=f"v{layer}_{t}", tag="v")
                nc.vector.tensor_mul(v_t, r_t, u_t)
                w_t = gs.tile([H, NL], BF16, name=f"w{layer}_{t}", tag="w")
                nc.vector.tensor_add(w_t, v_t, gn)
                n_t = gs.tile([H, NL], BF16, name=f"n{layer}_{t}", tag="n")
                nc.scalar.activation(n_t, w_t, AF.Tanh)
                d_t = gs.tile([H, NL], BF16, name=f"d{layer}_{t}", tag="d")
                nc.vector.tensor_sub(d_t, h_prev, n_t)
                e_t = gs.tile([H, NL], BF16, name=f"e{layer}_{t}", tag="e")
                nc.vector.tensor_mul(e_t, z_t, d_t)
                h_new = gs.tile([H, NL], BF16, name=f"h{layer}_{t}",
                                tag=f"h{layer}")
                nc.vector.tensor_add(h_new, n_t, e_t)
                return h_new

            h1 = gs.tile([H, NL], BF16, name="h1_init", tag="h0")
            nc.vector.memset(h1, 0.0)
            h2 = gs.tile([H, NL], BF16, name="h2_init", tag="h0")
            nc.vector.memset(h2, 0.0)

            for t in range(T):
                h1 = gru_step(0, t, h1, gi0rz[:, t, :], gi0n[:, t, :])
                # layer-1 input gates from ys1[t]
                pg = psum(f"pg{t}", [128, NL])
                nc.tensor.matmul(pg, lhsT=s_wih[1][:, 0:128], rhs=h1,
                                 start=True, stop=True)
                g1rz = gs.tile([2 * H, NL], BF16, name=f"g1rz{t}", tag="g1rz")
                nc.scalar.activation(g1rz, pg, AF.Identity, bias=s_girzb[1])
                pg2 = psum(f"pg2{t}", [H, NL])
                nc.tensor.matmul(pg2, lhsT=s_wih[1][:, 128:192], rhs=h1,
                                 start=True, stop=True)
                g1n = gs.tile([H, NL], BF16, name=f"g1n{t}", tag="g1n")
                nc.scalar.activation(g1n, pg2, AF.Identity, bias=s_ginb[1])
                h2 = gru_step(1, t, h2, g1rz, g1n)

            # ---- head: BN+ReLU, lin_out, log_softmax
            hl = pb.tile([H + 1, NL], BF16, name="hl", tag="hl")
            nc.scalar.activation(hl[0:64, :], h2, AF.Relu,
                                 bias=s_bout, scale=s_scout)
            nc.vector.memset(hl[64:65, :], 1.0)
            plg = psum("plg", [C, NL])
            nc.tensor.matmul(plg, lhsT=s_loutw, rhs=hl, start=True, stop=True)
            lg = pb.tile([C, NL], F32, name="lg", tag="lg")
            nc.vector.tensor_copy(lg, plg)
            for c2 in range(2):
                cn = 128 if c2 == 0 else NL - 128
                pt = psum(f"pt{c2}", [128, C])
                nc.tensor.transpose(pt[0:cn, :],
                                    lg[:, c2 * 128:c2 * 128 + cn],
                                    id_f32[0:C, 0:C])
                mx = pb.tile([128, 1], F32, name=f"mx{c2}", tag="mx", bufs=2)
                nc.vector.reduce_max(out=mx[0:cn, :], in_=pt[0:cn, :],
                                     axis=mybir.AxisListType.X, negate=True)
                ex = pb.tile([128, C], F32, name=f"ex{c2}", tag="ex", bufs=2)
                nc.scalar.activation(ex[0:cn, :], pt[0:cn, :], AF.Exp,
                                     bias=mx[0:cn, :])
                sm = pb.tile([128, 1], F32, name=f"sm{c2}", tag="sm", bufs=2)
                nc.vector.reduce_sum(out=sm[0:cn, :], in_=ex[0:cn, :],
                                     axis=mybir.AxisListType.X)
                ln = pb.tile([128, 1], F32, name=f"ln{c2}", tag="ln", bufs=2)
                nc.scalar.activation(ln[0:cn, :], sm[0:cn, :], AF.Ln)
                b2 = pb.tile([128, 1], F32, name=f"b2{c2}", tag="b2", bufs=2)
                nc.vector.tensor_sub(b2[0:cn, :], mx[0:cn, :], ln[0:cn, :])
                osb = pb.tile([128, C], F32, name=f"osb{c2}", tag="osb",
                              bufs=2)
                nc.scalar.activation(osb[0:cn, :], pt[0:cn, :], AF.Identity,
                                     bias=b2[0:cn, :])
                nc.sync.dma_start(out=out[c2 * 128:c2 * 128 + cn, :],
                                  in_=osb[0:cn, :])


def _build():
    if "nc" in _CACHE:
        return _CACHE["nc"]
    nc = bacc.Bacc("TRN2", target_bir_lowering=False, debug=False,
                   enable_asserts=False, num_devices=W)
    with tile.TileContext(nc) as tc:
        _emit(nc, tc)
    nc.compile()
    _CACHE["nc"] = nc
    return nc


def kernel_run(inputs, trace=False):
    nc = _build()
    in_maps = _prep(inputs)
    res = run_bass_kernel_spmd(nc, in_maps, core_ids=list(range(W)),
                               trace=trace)
    outs = np.concatenate(
        [np.asarray(res.results[c]["out"], np.float32) for c in range(W)],
        axis=0)
    return outs, res


def kernel(**inputs) -> np.ndarray:
    outs, _ = kernel_run(inputs, trace=False)
    return outs


# revision 14
# speedup vs baseline: 1.1979x; 1.1979x over previous
"""Trainium2 Bass kernel: NeuralGrangerCausality (GCN + causal attention + GRU).

Strategy (8 NeuronCores, SPMD):
  - Phase A (T-sharded, 8 timesteps/core): lin_in -> causal softmax-matmul
    -> fusion -> GCN x2 (dense normalized-adjacency matmul, built on host)
    all as bf16 PE matmuls with fp32 PSUM accumulation.
    Per-timestep node tensors are kept feature-major [H, N]; aggregation
    matmuls contract over nodes with 2-timestep-stacked lhsT ([j, 2t*h]) for
    full 128-wide PE utilization.
  - Reshard T->N via AllToAll (bf16, 2MB).
  - Phase B (N-sharded, 250 nodes/core): 2-layer GRU recurrence (input-gate
    matmuls batched, per-step Whh matmuls + gate math), BN+ReLU, lin_out,
    log_softmax.

kernel(**inputs) takes the FULL inputs, preps/shards on host (numpy only:
index->dense adjacency, transposes, BN folding), runs the NEFF on cores 0-7
via run_bass_kernel_spmd, and concatenates the per-core [250, 10] outputs.
"""

import os
import sys

import numpy as np

for _p in ("/opt/trn_rl_repo", "/root/.axon_site/_ro/trn_rl_repo"):
    if os.path.isdir(_p) and _p not in sys.path:
        sys.path.append(_p)

import ml_dtypes  # noqa: E402

import concourse.bass as bass  # noqa: E402,F401
import concourse.mybir as mybir  # noqa: E402
import concourse.tile as tile  # noqa: E402
from concourse import bacc  # noqa: E402
from concourse.bass_utils import run_bass_kernel_spmd  # noqa: E402
from concourse.masks import make_identity  # noqa: E402

AF = mybir.ActivationFunctionType
F32 = mybir.dt.float32
BF16 = mybir.dt.bfloat16
BF = ml_dtypes.bfloat16

T, N, F_IN, H, C = 64, 2000, 32, 64, 10
W = 8            # cores
TL = T // W      # timesteps per core (phase A)
NL = N // W      # nodes per core (phase B)
EPS = 1e-5
NJ = (N + 127) // 128                       # node contraction chunks
JCH = [(j * 128, min(128, N - j * 128)) for j in range(NJ)]
NI = 4
IC = N // NI                                # 500-wide free-dim chunks
ICH = [(i * IC, IC) for i in range(NI)]

_CACHE: dict = {}


# --------------------------------------------------------------------------
# host-side prep
# --------------------------------------------------------------------------

def _prep(inputs):
    f32 = np.float32
    g = {k: np.asarray(v) for k, v in inputs.items()}

    x_seq = g["x_seq"].astype(f32)                       # [T, N, F]
    src = g["edge_index"][0].astype(np.int64)
    dst = g["edge_index"][1].astype(np.int64)
    ew = g["edge_weight"].astype(f32)

    # GCN normalization with self loops (PyG gcn_norm, fill 1)
    loops = np.arange(N, dtype=np.int64)
    src_f = np.concatenate([src, loops])
    dst_f = np.concatenate([dst, loops])
    w_f = np.concatenate([ew, np.ones(N, f32)])
    deg = np.zeros(N, f32)
    np.add.at(deg, dst_f, w_f)
    dis = (1.0 / np.sqrt(np.maximum(deg, 1e-12))).astype(f32)
    norm = dis[src_f] * w_f * dis[dst_f]
    at = np.zeros((N, N), f32)                           # at[j, i] = A[i, j]
    np.add.at(at, (src_f, dst_f), norm)

    # x_seq feature-major with an appended ones-row (bias via matmul aug)
    xs = np.concatenate(
        [x_seq.transpose(0, 2, 1), np.ones((T, 1, N), f32)], axis=1
    )                                                    # [T, F+1, N]

    cwt = np.ascontiguousarray(g["causal_weight"].astype(f32).T)  # [j, i]

    linw = np.concatenate(
        [g["lin_in_w"].astype(f32).T, g["lin_in_b"].astype(f32)[None]], axis=0
    )                                                    # [F+1, H]

    fw = g["fusion_w"].astype(f32)                       # [H, 2H]
    fusw1d = np.concatenate([fw[:, :H].T, fw[:, :H].T], 0)   # [2H, H]
    fusw2d = np.concatenate([fw[:, H:].T, fw[:, H:].T], 0)
    fusbd = np.tile(g["fusion_b"].astype(f32), 2)[:, None]   # [2H, 1]

    def gcn_fold(wk, bk, gk, bbk, mk, vk):
        sc = g[gk].astype(f32) / np.sqrt(g[vk].astype(f32) + EPS)
        wt = g[wk].astype(f32).T * sc[None, :]           # [H_in, H_out]
        bias = (g[bk].astype(f32) - g[mk].astype(f32)) * sc + g[bbk].astype(f32)
        return (np.concatenate([wt, wt], 0),
                np.tile(bias, 2)[:, None])

    w0d, b0d = gcn_fold("gcn_w0", "gcn_b0", "bn0_g", "bn0_b", "bn0_m", "bn0_v")
    w1d, b1d = gcn_fold("gcn_w1", "gcn_b1", "bn1_g", "bn1_b", "bn1_m", "bn1_v")

    p = {
        "cwt": cwt, "at": at.astype(BF),
        "linw": linw.astype(BF),
        "fusw1d": fusw1d.astype(BF), "fusw2d": fusw2d.astype(BF),
        "fusbd": fusbd,
        "w0d": w0d.astype(BF), "b0d": b0d,
        "w1d": w1d.astype(BF), "b1d": b1d,
    }
    for layer in (0, 1):
        wih = g[f"gru_wih{layer}"].astype(f32)            # [3H, H]
        whh = g[f"gru_whh{layer}"].astype(f32)
        bih = g[f"gru_bih{layer}"].astype(f32)
        bhh = g[f"gru_bhh{layer}"].astype(f32)
        p[f"wih{layer}"] = wih.T.astype(BF)               # [H, 3H]
        p[f"whh{layer}"] = whh.T.astype(BF)
    p["girzb0"] = (g["gru_bih0"].astype(f32)
                   + g["gru_bhh0"].astype(f32))[: 2 * H, None]
    p["ginb0"] = g["gru_bih0"].astype(f32)[2 * H:, None]
    # rank-1 bias rows (bf16 lhsT for K=1 bias-injection matmuls)
    p["bhnb0_row"] = g["gru_bhh0"].astype(f32)[None, 2 * H:].astype(BF)
    p["bhnb1_row"] = g["gru_bhh1"].astype(f32)[None, 2 * H:].astype(BF)
    p["rzb1_row"] = (g["gru_bih1"].astype(f32)
                     + g["gru_bhh1"].astype(f32))[None, : 2 * H].astype(BF)
    p["ginb1_row"] = g["gru_bih1"].astype(f32)[None, 2 * H:].astype(BF)

    scout = g["bnout_g"].astype(f32) / np.sqrt(g["bnout_v"].astype(f32) + EPS)
    p["scout"] = scout[:, None]
    p["bout"] = (g["bnout_b"].astype(f32)
                 - g["bnout_m"].astype(f32) * scout)[:, None]
    p["loutw"] = np.concatenate(
        [g["lin_out_w"].astype(f32).T, g["lin_out_b"].astype(f32)[None]], 0
    ).astype(BF)                                          # [H+1, C]

    xs_bf = xs.astype(BF)
    in_maps = []
    for c in range(W):
        m = dict(p)
        m["xs"] = np.ascontiguousarray(xs_bf[c * TL:(c + 1) * TL])
        in_maps.append(m)
    return in_maps


# --------------------------------------------------------------------------
# kernel IR
# --------------------------------------------------------------------------

def _emit(nc, tc):
    def param(name, shape, dt):
        return nc.dram_tensor(name, shape, dt, kind="ExternalInput").ap()

    xs = param("xs", [TL, F_IN + 1, N], BF16)
    cwt = param("cwt", [N, N], F32)
    at = param("at", [N, N], BF16)
    linw = param("linw", [F_IN + 1, H], BF16)
    fusw1d = param("fusw1d", [2 * H, H], BF16)
    fusw2d = param("fusw2d", [2 * H, H], BF16)
    fusbd = param("fusbd", [2 * H, 1], F32)
    w0d = param("w0d", [2 * H, H], BF16)
    b0d = param("b0d", [2 * H, 1], F32)
    w1d = param("w1d", [2 * H, H], BF16)
    b1d = param("b1d", [2 * H, 1], F32)
    wih = [param(f"wih{l}", [H, 3 * H], BF16) for l in (0, 1)]
    whh = [param(f"whh{l}", [H, 3 * H], BF16) for l in (0, 1)]
    girzb0 = param("girzb0", [2 * H, 1], F32)
    ginb0 = param("ginb0", [H, 1], F32)
    bhnb0_row = param("bhnb0_row", [1, H], BF16)
    bhnb1_row = param("bhnb1_row", [1, H], BF16)
    rzb1_row = param("rzb1_row", [1, 2 * H], BF16)
    ginb1_row = param("ginb1_row", [1, H], BF16)
    scout = param("scout", [H, 1], F32)
    bout = param("bout", [H, 1], F32)
    loutw = param("loutw", [H + 1, C], BF16)
    out = nc.dram_tensor("out", [NL, C], F32, kind="ExternalOutput").ap()

    with tc.tile_pool(name="consts", bufs=1) as cst, \
         tc.tile_pool(name="dram", bufs=1, space="DRAM") as dram, \
         tc.tile_pool(name="ps", bufs=8, space="PSUM") as ps:

        def psum(pn, pshape):
            return ps.tile(pshape, F32, tag="ps", name=pn,
                           padded_shape=[128, 512])

        # ---- constants into SBUF
        def cload(ap_, cn):
            t_ = cst.tile(list(ap_.shape), ap_.dtype, name=cn, tag=cn)
            nc.sync.dma_start(out=t_, in_=ap_)
            return t_

        s_linw = cload(linw, "s_linw")
        s_fusw1 = cload(fusw1d, "s_fusw1")
        s_fusw2 = cload(fusw2d, "s_fusw2")
        s_fusb = cload(fusbd, "s_fusb")
        s_w0 = cload(w0d, "s_w0")
        s_b0 = cload(b0d, "s_b0")
        s_w1 = cload(w1d, "s_w1")
        s_b1 = cload(b1d, "s_b1")
        s_wih = [cload(wih[l], f"s_wih{l}") for l in (0, 1)]
        s_whh = [cload(whh[l], f"s_whh{l}") for l in (0, 1)]
        s_girzb0 = cload(girzb0, "s_girzb0")
        s_ginb0 = cload(ginb0, "s_ginb0")
        s_bhnb0r = cload(bhnb0_row, "s_bhnb0r")
        s_bhnb1r = cload(bhnb1_row, "s_bhnb1r")
        s_rzb1r = cload(rzb1_row, "s_rzb1r")
        s_ginb1r = cload(ginb1_row, "s_ginb1r")
        s_scout = cload(scout, "s_scout")
        s_bout = cload(bout, "s_bout")
        s_loutw = cload(loutw, "s_loutw")

        ones_col = cst.tile([128, 1], BF16, name="ones_col", tag="ones_col")
        nc.vector.memset(ones_col, 1.0)
        ones_row = cst.tile([1, NL], BF16, name="ones_row", tag="ones_row")
        nc.vector.memset(ones_row, 1.0)
        id_bf = cst.tile([128, 128], BF16, name="id_bf", tag="id_bf")
        make_identity(nc, id_bf)
        id_f32 = cst.tile([16, 16], F32, name="id_f32", tag="id_f32")
        make_identity(nc, id_f32)

        a2a_in = dram.tile([W, TL, H, NL], BF16, name="a2a_in", tag="a2a_in")
        a2a_out = dram.tile([W, TL, H, NL], BF16, name="a2a_out",
                            tag="a2a_out")

        # ================= PHASE A (T-sharded) =================
        with tc.tile_pool(name="pa", bufs=1) as pa, \
             tc.tile_pool(name="st", bufs=3) as st:

            # big per-t-pair stacked tiles
            x1nm = [pa.tile([128, NJ, 128], BF16, name=f"x1nm{q}",
                            tag=f"x1nm{q}") for q in range(TL // 2)]
            x1p = [pa.tile([128, N], BF16, name=f"x1p{q}", tag=f"x1p{q}")
                   for q in range(TL // 2)]
            xap = [pa.tile([128, N], BF16, name=f"xap{q}", tag=f"xap{q}")
                   for q in range(TL // 2)]
            x2p = [pa.tile([128, N], BF16, name=f"x2p{q}", tag=f"x2p{q}")
                   for q in range(TL // 2)]
            x3p = [pa.tile([128, N], BF16, name=f"x3p{q}", tag=f"x3p{q}")
                   for q in range(TL // 2)]
            x4p = [pa.tile([128, N], BF16, name=f"x4p{q}", tag=f"x4p{q}")
                   for q in range(TL // 2)]

            # ---- stage 1: x1 = relu(lin_in(x)) in both layouts
            for t in range(TL):
                q, o = t // 2, t % 2
                hs = slice(64 * o, 64 * o + 64)
                xst = st.tile([F_IN + 1, N], BF16, name=f"xs{t}", tag="xs")
                nc.sync.dma_start(out=xst, in_=xs[t])
                # feature-major into x1p halves
                for i, (i0, iw) in enumerate(ICH):
                    pfm = psum(f"pfm{t}_{i}", [128, IC])
                    nc.tensor.matmul(pfm[hs, :], lhsT=s_linw,
                                     rhs=xst[:, i0:i0 + iw],
                                     start=True, stop=True)
                    nc.scalar.activation(x1p[q][hs, i0:i0 + iw], pfm[hs, :],
                                         AF.Relu)
                # node-major into x1nm column halves
                for j, (j0, pj) in enumerate(JCH):
                    pnm = psum(f"pnm{t}_{j}", [128, H])
                    nc.tensor.matmul(pnm[0:pj, :], lhsT=xst[:, j0:j0 + pj],
                                     rhs=s_linw, start=True, stop=True)
                    nc.scalar.activation(
                        x1nm[q][0:pj, j, 64 * o:64 * o + 64],
                        pnm[0:pj, :], AF.Relu)

            # ---- stage 2: x_agg = softmax(cw) @ x1  (+ column rescale)
            for i, (i0, iw) in enumerate(ICH):
                pcs = psum(f"pcs{i}", [1, IC])
                pagg = [psum(f"pagg{i}_{q}", [128, IC])
                        for q in range(TL // 2)]
                for j, (j0, pj) in enumerate(JCH):
                    cwf = st.tile([128, IC], F32, name=f"cwf{i}_{j}",
                                  tag="cwf")
                    nc.sync.dma_start(out=cwf[0:pj, :],
                                      in_=cwt[j0:j0 + pj, i0:i0 + iw])
                    wct = st.tile([128, IC], BF16, name=f"wct{i}_{j}",
                                  tag="wct")
                    nc.scalar.activation(wct[0:pj, :], cwf[0:pj, :], AF.Exp)
                    nc.tensor.matmul(pcs, lhsT=ones_col[0:pj, :],
                                     rhs=wct[0:pj, :],
                                     start=(j == 0), stop=(j == NJ - 1))
                    for q in range(TL // 2):
                        nc.tensor.matmul(pagg[q],
                                         lhsT=x1nm[q][0:pj, j, :],
                                         rhs=wct[0:pj, :],
                                         start=(j == 0), stop=(j == NJ - 1))
                rinv = st.tile([1, IC], F32, name=f"rinv{i}", tag="rinv",
                               bufs=2)
                nc.vector.reciprocal(rinv, pcs)
                rb = st.tile([128, IC], F32, name=f"rb{i}", tag="rb", bufs=2)
                nc.gpsimd.partition_broadcast(rb, rinv)
                for q in range(TL // 2):
                    for o in (0, 1):
                        hs = slice(64 * o, 64 * o + 64)
                        nc.vector.tensor_mul(xap[q][hs, i0:i0 + iw],
                                             pagg[q][hs, :], rb[hs, :])

            # ---- stage 3: fusion x2 = relu(W1@x1 + W2@xagg + b)
            for q in range(TL // 2):
                for i, (i0, iw) in enumerate(ICH):
                    pf = psum(f"pf{q}_{i}", [128, IC])
                    for o in (0, 1):
                        hs = slice(64 * o, 64 * o + 64)
                        nc.tensor.matmul(pf[hs, :], lhsT=s_fusw1[hs, :],
                                         rhs=x1p[q][hs, i0:i0 + iw],
                                         start=True, stop=False)
                        nc.tensor.matmul(pf[hs, :], lhsT=s_fusw2[hs, :],
                                         rhs=xap[q][hs, i0:i0 + iw],
                                         start=False, stop=True)
                        nc.scalar.activation(x2p[q][hs, i0:i0 + iw],
                                             pf[hs, :], AF.Relu,
                                             bias=s_fusb[hs, :])

            # ---- GCN layer: z = (x @ Wsc) node-major, agg = A @ z, BN+ReLU
            def gcn_layer(xin, w_dup, b_dup, xout, evac):
                znm = [pa.tile([128, NJ, 128], BF16, name=f"znm{q}",
                               tag=f"znm{q}", bufs=2)
                       for q in range(TL // 2)]
                for t in range(TL):
                    q, o = t // 2, t % 2
                    hs = slice(64 * o, 64 * o + 64)
                    for j, (j0, pj) in enumerate(JCH):
                        pz = psum(f"pz{t}_{j}", [128, H])
                        nc.tensor.matmul(pz[0:pj, :],
                                         lhsT=xin[q][hs, j0:j0 + pj],
                                         rhs=w_dup[hs, :],
                                         start=True, stop=True)
                        nc.vector.tensor_copy(
                            znm[q][0:pj, j, 64 * o:64 * o + 64], pz[0:pj, :])
                for i, (i0, iw) in enumerate(ICH):
                    pagg = [psum(f"pag{i}_{q}", [128, IC])
                            for q in range(TL // 2)]
                    for j, (j0, pj) in enumerate(JCH):
                        atb = st.tile([128, IC], BF16, name=f"atb{i}_{j}",
                                      tag="atb", bufs=4)
                        nc.sync.dma_start(out=atb[0:pj, :],
                                          in_=at[j0:j0 + pj, i0:i0 + iw])
                        for q in range(TL // 2):
                            nc.tensor.matmul(pagg[q],
                                             lhsT=znm[q][0:pj, j, :],
                                             rhs=atb[0:pj, :],
                                             start=(j == 0),
                                             stop=(j == NJ - 1))
                    for q in range(TL // 2):
                        for o in (0, 1):
                            hs = slice(64 * o, 64 * o + 64)
                            evac(pagg[q], q, o, hs, i0, iw, b_dup, xout)

            def evac_bn(pagg, q, o, hs, i0, iw, b_dup, xout):
                nc.scalar.activation(xout[q][hs, i0:i0 + iw], pagg[hs, :],
                                     AF.Relu, bias=b_dup[hs, :])

            gcn_layer(x2p, s_w0, s_b0, x3p, evac_bn)
            gcn_layer(x3p, s_w1, s_b1, x4p, evac_bn)

            # ---- ship x4 to the all-to-all buffer (t-pair per DMA)
            for q in range(TL // 2):
                for d in range(W):
                    nc.sync.dma_start(
                        out=a2a_in[d, 2 * q:2 * q + 2],
                        in_=x4p[q][:, d * NL:(d + 1) * NL])

        # ================= reshard T -> N =================
        nc.gpsimd.collective_compute(
            "AllToAll", mybir.AluOpType.bypass,
            replica_groups=[list(range(W))],
            ins=[a2a_in.opt()], outs=[a2a_out.opt()])

        # ================= PHASE B (N-sharded GRU) =================
        with tc.tile_pool(name="pb", bufs=1) as pb, \
             tc.tile_pool(name="gs", bufs=4) as gs:

            x4all = pb.tile([H, W, TL, NL], BF16, name="x4all", tag="x4all")
            nc.sync.dma_start(out=x4all,
                              in_=a2a_out.rearrange("s t h n -> h s t n"))
            x4f = x4all.rearrange("h s t n -> h (s t n)")

            # layer-0 input gates, batched over pairs of timesteps
            gi0rz = pb.tile([2 * H, T, NL], BF16, name="gi0rz", tag="gi0rz")
            gi0n = pb.tile([H, T, NL], BF16, name="gi0n", tag="gi0n")
            gi0rzf = gi0rz.rearrange("p t n -> p (t n)")
            gi0nf = gi0n.rearrange("p t n -> p (t n)")
            for p2 in range(T // 2):
                csl = slice(2 * p2 * NL, (2 * p2 + 2) * NL)
                rhs = x4f[:, csl]
                prz = psum(f"prz{p2}", [128, 2 * NL])
                nc.tensor.matmul(prz, lhsT=s_wih[0][:, 0:128], rhs=rhs,
                                 start=True, stop=True)
                nc.scalar.activation(gi0rzf[:, csl], prz,
                                     AF.Identity, bias=s_girzb0)
                pn = psum(f"pn{p2}", [H, 2 * NL])
                nc.tensor.matmul(pn, lhsT=s_wih[0][:, 128:192], rhs=rhs,
                                 start=True, stop=True)
                nc.scalar.activation(gi0nf[:, csl], pn,
                                     AF.Identity, bias=s_ginb0)

            def gate_tail(layer, t, h_prev, prz, pu, w_in2):
                """sigmoid/tanh gate math; w_in2 is the gi_n operand
                (SBUF slice for L1, PSUM bank for L2)."""
                r_t = gs.tile([H, NL], BF16, name=f"r{layer}_{t}", tag="r")
                nc.scalar.activation(r_t, prz[0:64, :], AF.Sigmoid)
                z_t = gs.tile([H, NL], BF16, name=f"z{layer}_{t}", tag="z")
                nc.scalar.activation(z_t, prz[64:128, :], AF.Sigmoid)
                v_t = gs.tile([H, NL], BF16, name=f"v{layer}_{t}", tag="v")
                nc.vector.tensor_mul(v_t, r_t, pu)
                w_t = gs.tile([H, NL], BF16, name=f"w{layer}_{t}", tag="w")
                nc.vector.tensor_add(w_t, v_t, w_in2)
                n_t = gs.tile([H, NL], BF16, name=f"n{layer}_{t}", tag="n")
                nc.scalar.activation(n_t, w_t, AF.Tanh)
                d_t = gs.tile([H, NL], BF16, name=f"d{layer}_{t}", tag="d")
                nc.vector.tensor_sub(d_t, h_prev, n_t)
                e_t = gs.tile([H, NL], BF16, name=f"e{layer}_{t}", tag="e")
                nc.vector.tensor_mul(e_t, z_t, d_t)
                h_new = gs.tile([H, NL], BF16, name=f"h{layer}_{t}",
                                tag=f"h{layer}")
                nc.vector.tensor_add(h_new, n_t, e_t)
                return h_new

            h1 = gs.tile([H, NL], BF16, name="h1_init", tag="h0")
            nc.vector.memset(h1, 0.0)
            h2 = gs.tile([H, NL], BF16, name="h2_init", tag="h0")
            nc.vector.memset(h2, 0.0)

            for t in range(T):
                # layer 0: gi precomputed in SBUF, injected via identity mm
                prz = psum(f"prza{t}", [128, NL])
                nc.tensor.matmul(prz, lhsT=id_bf, rhs=gi0rz[:, t, :],
                                 start=True, stop=False)
                nc.tensor.matmul(prz, lhsT=s_whh[0][:, 0:128], rhs=h1,
                                 start=False, stop=True)
                pu = psum(f"pua{t}", [H, NL])
                nc.tensor.matmul(pu, lhsT=s_bhnb0r, rhs=ones_row,
                                 start=True, stop=False)
                nc.tensor.matmul(pu, lhsT=s_whh[0][:, 128:192], rhs=h1,
                                 start=False, stop=True)
                h1 = gate_tail(0, t, h1, prz, pu, gi0n[:, t, :])

                # layer 1: input gates + biases accumulated straight in PSUM
                prz2 = psum(f"przb{t}", [128, NL])
                nc.tensor.matmul(prz2, lhsT=s_rzb1r, rhs=ones_row,
                                 start=True, stop=False)
                nc.tensor.matmul(prz2, lhsT=s_wih[1][:, 0:128], rhs=h1,
                                 start=False, stop=False)
                nc.tensor.matmul(prz2, lhsT=s_whh[1][:, 0:128], rhs=h2,
                                 start=False, stop=True)
                pb2 = psum(f"pgb{t}", [H, NL])
                nc.tensor.matmul(pb2, lhsT=s_ginb1r, rhs=ones_row,
                                 start=True, stop=False)
                nc.tensor.matmul(pb2, lhsT=s_wih[1][:, 128:192], rhs=h1,
                                 start=False, stop=True)
                pu2 = psum(f"pub{t}", [H, NL])
                nc.tensor.matmul(pu2, lhsT=s_bhnb1r, rhs=ones_row,
                                 start=True, stop=False)
                nc.tensor.matmul(pu2, lhsT=s_whh[1][:, 128:192], rhs=h2,
                                 start=False, stop=True)
                h2 = gate_tail(1, t, h2, prz2, pu2, pb2)

            # ---- head: BN+ReLU, lin_out, log_softmax
            hl = pb.tile([H + 1, NL], BF16, name="hl", tag="hl")
            nc.scalar.activation(hl[0:64, :], h2, AF.Relu,
                                 bias=s_bout, scale=s_scout)
            nc.vector.memset(hl[64:65, :], 1.0)
            plg = psum("plg", [C, NL])
            nc.tensor.matmul(plg, lhsT=s_loutw, rhs=hl, start=True, stop=True)
            lg = pb.tile([C, NL], F32, name="lg", tag="lg")
            nc.vector.tensor_copy(lg, plg)
            for c2 in range(2):
                cn = 128 if c2 == 0 else NL - 128
                pt = psum(f"pt{c2}", [128, C])
                nc.tensor.transpose(pt[0:cn, :],
                                    lg[:, c2 * 128:c2 * 128 + cn],
                                    id_f32[0:C, 0:C])
                mx = pb.tile([128, 1], F32, name=f"mx{c2}", tag="mx", bufs=2)
                nc.vector.reduce_max(out=mx[0:cn, :], in_=pt[0:cn, :],
                                     axis=mybir.AxisListType.X, negate=True)
                ex = pb.tile([128, C], F32, name=f"ex{c2}", tag="ex", bufs=2)
                nc.scalar.activation(ex[0:cn, :], pt[0:cn, :], AF.Exp,
                                     bias=mx[0:cn, :])
                sm = pb.tile([128, 1], F32, name=f"sm{c2}", tag="sm", bufs=2)
                nc.vector.reduce_sum(out=sm[0:cn, :], in_=ex[0:cn, :],
                                     axis=mybir.AxisListType.X)
                ln = pb.tile([128, 1], F32, name=f"ln{c2}", tag="ln", bufs=2)
                nc.scalar.activation(ln[0:cn, :], sm[0:cn, :], AF.Ln)
                b2 = pb.tile([128, 1], F32, name=f"b2{c2}", tag="b2", bufs=2)
                nc.vector.tensor_sub(b2[0:cn, :], mx[0:cn, :], ln[0:cn, :])
                osb = pb.tile([128, C], F32, name=f"osb{c2}", tag="osb",
                              bufs=2)
                nc.scalar.activation(osb[0:cn, :], pt[0:cn, :], AF.Identity,
                                     bias=b2[0:cn, :])
                nc.sync.dma_start(out=out[c2 * 128:c2 * 128 + cn, :],
                                  in_=osb[0:cn, :])


def _build():
    if "nc" in _CACHE:
        return _CACHE["nc"]
    nc = bacc.Bacc("TRN2", target_bir_lowering=False, debug=False,
                   enable_asserts=False, num_devices=W)
    with tile.TileContext(nc) as tc:
        _emit(nc, tc)
    nc.compile()
    _CACHE["nc"] = nc
    return nc


def kernel_run(inputs, trace=False):
    nc = _build()
    in_maps = _prep(inputs)
    res = run_bass_kernel_spmd(nc, in_maps, core_ids=list(range(W)),
                               trace=trace)
    outs = np.concatenate(
        [np.asarray(res.results[c]["out"], np.float32) for c in range(W)],
        axis=0)
    return outs, res


def kernel(**inputs) -> np.ndarray:
    outs, _ = kernel_run(inputs, trace=False)
    return outs


# revision 16
# speedup vs baseline: 1.2445x; 1.0389x over previous
"""Trainium2 Bass kernel: NeuralGrangerCausality (GCN + causal attention + GRU).

Strategy (8 NeuronCores, SPMD):
  - Phase A (T-sharded, 8 timesteps/core): lin_in -> causal softmax-matmul
    -> fusion -> GCN x2 (dense normalized-adjacency matmul, built on host)
    all as bf16 PE matmuls with fp32 PSUM accumulation.
    Per-timestep node tensors are kept feature-major [H, N]; aggregation
    matmuls contract over nodes with 2-timestep-stacked lhsT ([j, 2t*h]) for
    full 128-wide PE utilization.
  - Reshard T->N via AllToAll (bf16, 2MB).
  - Phase B (N-sharded, 250 nodes/core): 2-layer GRU recurrence (input-gate
    matmuls batched, per-step Whh matmuls + gate math), BN+ReLU, lin_out,
    log_softmax.

kernel(**inputs) takes the FULL inputs, preps/shards on host (numpy only:
index->dense adjacency, transposes, BN folding), runs the NEFF on cores 0-7
via run_bass_kernel_spmd, and concatenates the per-core [250, 10] outputs.
"""

import os
import sys

import numpy as np

for _p in ("/opt/trn_rl_repo", "/root/.axon_site/_ro/trn_rl_repo"):
    if os.path.isdir(_p) and _p not in sys.path:
        sys.path.append(_p)

import ml_dtypes  # noqa: E402

import concourse.bass as bass  # noqa: E402,F401
import concourse.mybir as mybir  # noqa: E402
import concourse.tile as tile  # noqa: E402
from concourse import bacc  # noqa: E402
from concourse.bass_utils import run_bass_kernel_spmd  # noqa: E402
from concourse.masks import make_identity  # noqa: E402

AF = mybir.ActivationFunctionType
F32 = mybir.dt.float32
BF16 = mybir.dt.bfloat16
BF = ml_dtypes.bfloat16

T, N, F_IN, H, C = 64, 2000, 32, 64, 10
W = 8            # cores
TL = T // W      # timesteps per core (phase A)
NL = N // W      # nodes per core (phase B)
EPS = 1e-5
NJ = (N + 127) // 128                       # node contraction chunks
JCH = [(j * 128, min(128, N - j * 128)) for j in range(NJ)]
NI = 4
IC = N // NI                                # 500-wide free-dim chunks
ICH = [(i * IC, IC) for i in range(NI)]

_CACHE: dict = {}


# --------------------------------------------------------------------------
# host-side prep
# --------------------------------------------------------------------------

def _prep(inputs):
    f32 = np.float32
    g = {k: np.asarray(v) for k, v in inputs.items()}

    x_seq = g["x_seq"].astype(f32)                       # [T, N, F]
    src = g["edge_index"][0].astype(np.int64)
    dst = g["edge_index"][1].astype(np.int64)
    ew = g["edge_weight"].astype(f32)

    # GCN normalization with self loops (PyG gcn_norm, fill 1)
    loops = np.arange(N, dtype=np.int64)
    src_f = np.concatenate([src, loops])
    dst_f = np.concatenate([dst, loops])
    w_f = np.concatenate([ew, np.ones(N, f32)])
    deg = np.zeros(N, f32)
    np.add.at(deg, dst_f, w_f)
    dis = (1.0 / np.sqrt(np.maximum(deg, 1e-12))).astype(f32)
    norm = dis[src_f] * w_f * dis[dst_f]
    at = np.zeros((N, N), f32)                           # at[j, i] = A[i, j]
    np.add.at(at, (src_f, dst_f), norm)

    # x_seq feature-major with an appended ones-row (bias via matmul aug)
    xs = np.concatenate(
        [x_seq.transpose(0, 2, 1), np.ones((T, 1, N), f32)], axis=1
    )                                                    # [T, F+1, N]

    cwt = np.ascontiguousarray(g["causal_weight"].astype(f32).T)  # [j, i]

    linw = np.concatenate(
        [g["lin_in_w"].astype(f32).T, g["lin_in_b"].astype(f32)[None]], axis=0
    )                                                    # [F+1, H]

    fw = g["fusion_w"].astype(f32)                       # [H, 2H]
    fusw1d = np.concatenate([fw[:, :H].T, fw[:, :H].T], 0)   # [2H, H]
    fusw2d = np.concatenate([fw[:, H:].T, fw[:, H:].T], 0)
    fusbd = np.tile(g["fusion_b"].astype(f32), 2)[:, None]   # [2H, 1]

    def gcn_fold(wk, bk, gk, bbk, mk, vk):
        sc = g[gk].astype(f32) / np.sqrt(g[vk].astype(f32) + EPS)
        wt = g[wk].astype(f32).T * sc[None, :]           # [H_in, H_out]
        bias = (g[bk].astype(f32) - g[mk].astype(f32)) * sc + g[bbk].astype(f32)
        return (np.concatenate([wt, wt], 0),
                np.tile(bias, 2)[:, None])

    w0d, b0d = gcn_fold("gcn_w0", "gcn_b0", "bn0_g", "bn0_b", "bn0_m", "bn0_v")
    w1d, b1d = gcn_fold("gcn_w1", "gcn_b1", "bn1_g", "bn1_b", "bn1_m", "bn1_v")

    p = {
        "cwt": cwt, "at": at.astype(BF),
        "linw": linw.astype(BF),
        "fusw1d": fusw1d.astype(BF), "fusw2d": fusw2d.astype(BF),
        "fusbd": fusbd,
        "w0d": w0d.astype(BF), "b0d": b0d,
        "w1d": w1d.astype(BF), "b1d": b1d,
    }
    for layer in (0, 1):
        wih = g[f"gru_wih{layer}"].astype(f32)            # [3H, H]
        whh = g[f"gru_whh{layer}"].astype(f32)
        bih = g[f"gru_bih{layer}"].astype(f32)
        bhh = g[f"gru_bhh{layer}"].astype(f32)
        p[f"wih{layer}"] = wih.T.astype(BF)               # [H, 3H]
        p[f"whh{layer}"] = whh.T.astype(BF)
    p["girzb0"] = (g["gru_bih0"].astype(f32)
                   + g["gru_bhh0"].astype(f32))[: 2 * H, None]
    p["ginb0"] = g["gru_bih0"].astype(f32)[2 * H:, None]
    # rank-1 bias rows (bf16 lhsT for K=1 bias-injection matmuls)
    p["bhnb0_row"] = g["gru_bhh0"].astype(f32)[None, 2 * H:].astype(BF)
    p["bhnb1_row"] = g["gru_bhh1"].astype(f32)[None, 2 * H:].astype(BF)
    p["rzb1_row"] = (g["gru_bih1"].astype(f32)
                     + g["gru_bhh1"].astype(f32))[None, : 2 * H].astype(BF)
    p["ginb1_row"] = g["gru_bih1"].astype(f32)[None, 2 * H:].astype(BF)

    scout = g["bnout_g"].astype(f32) / np.sqrt(g["bnout_v"].astype(f32) + EPS)
    p["scout"] = scout[:, None]
    p["bout"] = (g["bnout_b"].astype(f32)
                 - g["bnout_m"].astype(f32) * scout)[:, None]
    p["loutw"] = np.concatenate(
        [g["lin_out_w"].astype(f32).T, g["lin_out_b"].astype(f32)[None]], 0
    ).astype(BF)                                          # [H+1, C]

    xs_bf = xs.astype(BF)
    in_maps = []
    for c in range(W):
        m = dict(p)
        m["xs"] = np.ascontiguousarray(xs_bf[c * TL:(c + 1) * TL])
        in_maps.append(m)
    return in_maps


# --------------------------------------------------------------------------
# kernel IR
# --------------------------------------------------------------------------

def _emit(nc, tc):
    def param(name, shape, dt):
        return nc.dram_tensor(name, shape, dt, kind="ExternalInput").ap()

    xs = param("xs", [TL, F_IN + 1, N], BF16)
    cwt = param("cwt", [N, N], F32)
    at = param("at", [N, N], BF16)
    linw = param("linw", [F_IN + 1, H], BF16)
    fusw1d = param("fusw1d", [2 * H, H], BF16)
    fusw2d = param("fusw2d", [2 * H, H], BF16)
    fusbd = param("fusbd", [2 * H, 1], F32)
    w0d = param("w0d", [2 * H, H], BF16)
    b0d = param("b0d", [2 * H, 1], F32)
    w1d = param("w1d", [2 * H, H], BF16)
    b1d = param("b1d", [2 * H, 1], F32)
    wih = [param(f"wih{l}", [H, 3 * H], BF16) for l in (0, 1)]
    whh = [param(f"whh{l}", [H, 3 * H], BF16) for l in (0, 1)]
    girzb0 = param("girzb0", [2 * H, 1], F32)
    ginb0 = param("ginb0", [H, 1], F32)
    bhnb0_row = param("bhnb0_row", [1, H], BF16)
    bhnb1_row = param("bhnb1_row", [1, H], BF16)
    rzb1_row = param("rzb1_row", [1, 2 * H], BF16)
    ginb1_row = param("ginb1_row", [1, H], BF16)
    scout = param("scout", [H, 1], F32)
    bout = param("bout", [H, 1], F32)
    loutw = param("loutw", [H + 1, C], BF16)
    out = nc.dram_tensor("out", [NL, C], F32, kind="ExternalOutput").ap()

    with tc.tile_pool(name="consts", bufs=1) as cst, \
         tc.tile_pool(name="dram", bufs=1, space="DRAM") as dram, \
         tc.tile_pool(name="ps", bufs=8, space="PSUM") as ps:

        def psum(pn, pshape):
            return ps.tile(pshape, F32, tag="ps", name=pn,
                           padded_shape=[128, 512])

        # ---- constants into SBUF
        def cload(ap_, cn):
            t_ = cst.tile(list(ap_.shape), ap_.dtype, name=cn, tag=cn)
            nc.sync.dma_start(out=t_, in_=ap_)
            return t_

        s_linw = cload(linw, "s_linw")
        s_fusw1 = cload(fusw1d, "s_fusw1")
        s_fusw2 = cload(fusw2d, "s_fusw2")
        s_fusb = cload(fusbd, "s_fusb")
        s_w0 = cload(w0d, "s_w0")
        s_b0 = cload(b0d, "s_b0")
        s_w1 = cload(w1d, "s_w1")
        s_b1 = cload(b1d, "s_b1")
        s_wih = [cload(wih[l], f"s_wih{l}") for l in (0, 1)]
        s_whh = [cload(whh[l], f"s_whh{l}") for l in (0, 1)]
        s_girzb0 = cload(girzb0, "s_girzb0")
        s_ginb0 = cload(ginb0, "s_ginb0")
        s_bhnb0r = cload(bhnb0_row, "s_bhnb0r")
        s_bhnb1r = cload(bhnb1_row, "s_bhnb1r")
        s_rzb1r = cload(rzb1_row, "s_rzb1r")
        s_ginb1r = cload(ginb1_row, "s_ginb1r")
        s_scout = cload(scout, "s_scout")
        s_bout = cload(bout, "s_bout")
        s_loutw = cload(loutw, "s_loutw")

        ones_col = cst.tile([128, 1], BF16, name="ones_col", tag="ones_col")
        nc.vector.memset(ones_col, 1.0)
        ones_row = cst.tile([1, NL], BF16, name="ones_row", tag="ones_row")
        nc.vector.memset(ones_row, 1.0)
        id_bf = cst.tile([128, 128], BF16, name="id_bf", tag="id_bf")
        make_identity(nc, id_bf)
        id_f32 = cst.tile([16, 16], F32, name="id_f32", tag="id_f32")
        make_identity(nc, id_f32)

        a2a_in = [dram.tile([W, 2, H, NL], BF16, name=f"a2a_in{q}",
                            tag=f"a2a_in{q}") for q in range(TL // 2)]
        a2a_out = [dram.tile([W, 2, H, NL], BF16, name=f"a2a_out{q}",
                             tag=f"a2a_out{q}") for q in range(TL // 2)]

        # ================= PHASE A (T-sharded) =================
        with tc.tile_pool(name="pa", bufs=1) as pa, \
             tc.tile_pool(name="st", bufs=3) as st:

            # big per-t-pair stacked tiles
            x1nm = [pa.tile([128, NJ, 128], BF16, name=f"x1nm{q}",
                            tag=f"x1nm{q}") for q in range(TL // 2)]
            x1p = [pa.tile([128, N], BF16, name=f"x1p{q}", tag=f"x1p{q}")
                   for q in range(TL // 2)]
            xap = [pa.tile([128, N], BF16, name=f"xap{q}", tag=f"xap{q}")
                   for q in range(TL // 2)]
            x2p = [pa.tile([128, N], BF16, name=f"x2p{q}", tag=f"x2p{q}")
                   for q in range(TL // 2)]
            x3p = [pa.tile([128, N], BF16, name=f"x3p{q}", tag=f"x3p{q}")
                   for q in range(TL // 2)]
            x4p = [pa.tile([128, N], BF16, name=f"x4p{q}", tag=f"x4p{q}")
                   for q in range(TL // 2)]

            # ---- stage 1: x1 = relu(lin_in(x)) in both layouts
            for t in range(TL):
                q, o = t // 2, t % 2
                hs = slice(64 * o, 64 * o + 64)
                xst = st.tile([F_IN + 1, N], BF16, name=f"xs{t}", tag="xs")
                nc.sync.dma_start(out=xst, in_=xs[t])
                # feature-major into x1p halves
                for i, (i0, iw) in enumerate(ICH):
                    pfm = psum(f"pfm{t}_{i}", [128, IC])
                    nc.tensor.matmul(pfm[hs, :], lhsT=s_linw,
                                     rhs=xst[:, i0:i0 + iw],
                                     start=True, stop=True)
                    nc.vector.tensor_scalar_max(x1p[q][hs, i0:i0 + iw],
                                                pfm[hs, :], 0.0)
                # node-major into x1nm column halves
                for j, (j0, pj) in enumerate(JCH):
                    pnm = psum(f"pnm{t}_{j}", [128, H])
                    nc.tensor.matmul(pnm[0:pj, :], lhsT=xst[:, j0:j0 + pj],
                                     rhs=s_linw, start=True, stop=True)
                    nc.vector.tensor_scalar_max(
                        x1nm[q][0:pj, j, 64 * o:64 * o + 64],
                        pnm[0:pj, :], 0.0)

            # ---- stage 2: x_agg = softmax(cw) @ x1  (+ column rescale)
            for i, (i0, iw) in enumerate(ICH):
                pcs = psum(f"pcs{i}", [1, IC])
                pagg = [psum(f"pagg{i}_{q}", [128, IC])
                        for q in range(TL // 2)]
                for j, (j0, pj) in enumerate(JCH):
                    cwf = st.tile([128, IC], F32, name=f"cwf{i}_{j}",
                                  tag="cwf")
                    nc.sync.dma_start(out=cwf[0:pj, :],
                                      in_=cwt[j0:j0 + pj, i0:i0 + iw])
                    wct = st.tile([128, IC], BF16, name=f"wct{i}_{j}",
                                  tag="wct")
                    nc.scalar.activation(wct[0:pj, :], cwf[0:pj, :], AF.Exp)
                    nc.tensor.matmul(pcs, lhsT=ones_col[0:pj, :],
                                     rhs=wct[0:pj, :],
                                     start=(j == 0), stop=(j == NJ - 1))
                    for q in range(TL // 2):
                        nc.tensor.matmul(pagg[q],
                                         lhsT=x1nm[q][0:pj, j, :],
                                         rhs=wct[0:pj, :],
                                         start=(j == 0), stop=(j == NJ - 1))
                rinv = st.tile([1, IC], F32, name=f"rinv{i}", tag="rinv",
                               bufs=2)
                nc.vector.reciprocal(rinv, pcs)
                rb = st.tile([128, IC], F32, name=f"rb{i}", tag="rb", bufs=2)
                nc.gpsimd.partition_broadcast(rb, rinv)
                for q in range(TL // 2):
                    for o in (0, 1):
                        hs = slice(64 * o, 64 * o + 64)
                        nc.vector.tensor_mul(xap[q][hs, i0:i0 + iw],
                                             pagg[q][hs, :], rb[hs, :])

            # ---- stage 3: fusion x2 = relu(W1@x1 + W2@xagg + b)
            for q in range(TL // 2):
                for i, (i0, iw) in enumerate(ICH):
                    pf = psum(f"pf{q}_{i}", [128, IC])
                    for o in (0, 1):
                        hs = slice(64 * o, 64 * o + 64)
                        nc.tensor.matmul(pf[hs, :], lhsT=s_fusw1[hs, :],
                                         rhs=x1p[q][hs, i0:i0 + iw],
                                         start=True, stop=False)
                        nc.tensor.matmul(pf[hs, :], lhsT=s_fusw2[hs, :],
                                         rhs=xap[q][hs, i0:i0 + iw],
                                         start=False, stop=True)
                        nc.scalar.activation(x2p[q][hs, i0:i0 + iw],
                                             pf[hs, :], AF.Relu,
                                             bias=s_fusb[hs, :])

            # ---- GCN layer: z = (x @ Wsc) node-major, agg = A @ z, BN+ReLU
            def gcn_layer(xin, w_dup, b_dup, xout, evac):
                znm = [pa.tile([128, NJ, 128], BF16, name=f"znm{q}",
                               tag=f"znm{q}", bufs=2)
                       for q in range(TL // 2)]
                for t in range(TL):
                    q, o = t // 2, t % 2
                    hs = slice(64 * o, 64 * o + 64)
                    for j, (j0, pj) in enumerate(JCH):
                        pz = psum(f"pz{t}_{j}", [128, H])
                        nc.tensor.matmul(pz[0:pj, :],
                                         lhsT=xin[q][hs, j0:j0 + pj],
                                         rhs=w_dup[hs, :],
                                         start=True, stop=True)
                        nc.vector.tensor_copy(
                            znm[q][0:pj, j, 64 * o:64 * o + 64], pz[0:pj, :])
                for i, (i0, iw) in enumerate(ICH):
                    pagg = [psum(f"pag{i}_{q}", [128, IC])
                            for q in range(TL // 2)]
                    for j, (j0, pj) in enumerate(JCH):
                        atb = st.tile([128, IC], BF16, name=f"atb{i}_{j}",
                                      tag="atb", bufs=4)
                        nc.sync.dma_start(out=atb[0:pj, :],
                                          in_=at[j0:j0 + pj, i0:i0 + iw])
                        for q in range(TL // 2):
                            nc.tensor.matmul(pagg[q],
                                             lhsT=znm[q][0:pj, j, :],
                                             rhs=atb[0:pj, :],
                                             start=(j == 0),
                                             stop=(j == NJ - 1))
                    for q in range(TL // 2):
                        for o in (0, 1):
                            hs = slice(64 * o, 64 * o + 64)
                            evac(pagg[q], q, o, hs, i0, iw, b_dup, xout)

            def evac_bn(pagg, q, o, hs, i0, iw, b_dup, xout):
                nc.scalar.activation(xout[q][hs, i0:i0 + iw], pagg[hs, :],
                                     AF.Relu, bias=b_dup[hs, :])

            gcn_layer(x2p, s_w0, s_b0, x3p, evac_bn)
            gcn_layer(x3p, s_w1, s_b1, x4p, evac_bn)

            # ---- ship x4 to the all-to-all buffers (t-pair per DMA),
            # one collective per t-pair so reshard overlaps the tail
            for q in range(TL // 2):
                for d in range(W):
                    nc.sync.dma_start(
                        out=a2a_in[q][d],
                        in_=x4p[q][:, d * NL:(d + 1) * NL])
                nc.gpsimd.collective_compute(
                    "AllToAll", mybir.AluOpType.bypass,
                    replica_groups=[list(range(W))],
                    ins=[a2a_in[q].opt()], outs=[a2a_out[q].opt()])

        # ================= PHASE B (N-sharded GRU) =================
        with tc.tile_pool(name="pb", bufs=1) as pb, \
             tc.tile_pool(name="gs", bufs=4) as gs:

            x4all = pb.tile([H, W, TL, NL], BF16, name="x4all", tag="x4all")
            for q in range(TL // 2):
                for sct in range(W):
                    nc.sync.dma_start(
                        out=x4all[:, sct, 2 * q:2 * q + 2, :],
                        in_=a2a_out[q][sct].rearrange("t h n -> h t n"))
            x4f = x4all.rearrange("h s t n -> h (s t n)")

            # layer-0 input gates, batched over pairs of timesteps
            gi0rz = pb.tile([2 * H, T, NL], BF16, name="gi0rz", tag="gi0rz")
            gi0n = pb.tile([H, T, NL], BF16, name="gi0n", tag="gi0n")
            gi0rzf = gi0rz.rearrange("p t n -> p (t n)")
            gi0nf = gi0n.rearrange("p t n -> p (t n)")
            for p2 in range(T // 2):
                csl = slice(2 * p2 * NL, (2 * p2 + 2) * NL)
                rhs = x4f[:, csl]
                prz = psum(f"prz{p2}", [128, 2 * NL])
                nc.tensor.matmul(prz, lhsT=s_wih[0][:, 0:128], rhs=rhs,
                                 start=True, stop=True)
                nc.scalar.activation(gi0rzf[:, csl], prz,
                                     AF.Identity, bias=s_girzb0)
                pn = psum(f"pn{p2}", [H, 2 * NL])
                nc.tensor.matmul(pn, lhsT=s_wih[0][:, 128:192], rhs=rhs,
                                 start=True, stop=True)
                nc.scalar.activation(gi0nf[:, csl], pn,
                                     AF.Identity, bias=s_ginb0)

            def gate_tail(layer, t, h_prev, prz, pu, w_in2):
                """sigmoid/tanh gate math; w_in2 is the gi_n operand
                (SBUF slice for L1, PSUM bank for L2)."""
                r_t = gs.tile([H, NL], BF16, name=f"r{layer}_{t}", tag="r")
                nc.scalar.activation(r_t, prz[0:64, :], AF.Sigmoid)
                z_t = gs.tile([H, NL], BF16, name=f"z{layer}_{t}", tag="z")
                nc.scalar.activation(z_t, prz[64:128, :], AF.Sigmoid)
                zp_t = gs.tile([H, NL], BF16, name=f"zp{layer}_{t}", tag="zp")
                nc.scalar.activation(zp_t, prz[64:128, :], AF.Sigmoid,
                                     scale=-1.0)
                zh_t = gs.tile([H, NL], BF16, name=f"zh{layer}_{t}", tag="zh")
                nc.vector.tensor_mul(zh_t, z_t, h_prev)
                v_t = gs.tile([H, NL], BF16, name=f"v{layer}_{t}", tag="v")
                nc.vector.tensor_mul(v_t, r_t, pu)
                w_t = gs.tile([H, NL], BF16, name=f"w{layer}_{t}", tag="w")
                nc.vector.tensor_add(w_t, v_t, w_in2)
                n_t = gs.tile([H, NL], BF16, name=f"n{layer}_{t}", tag="n")
                nc.scalar.activation(n_t, w_t, AF.Tanh)
                m_t = gs.tile([H, NL], BF16, name=f"m{layer}_{t}", tag="m")
                nc.vector.tensor_mul(m_t, n_t, zp_t)
                h_new = gs.tile([H, NL], BF16, name=f"h{layer}_{t}",
                                tag=f"h{layer}")
                nc.vector.tensor_add(h_new, m_t, zh_t)
                return h_new

            h1 = gs.tile([H, NL], BF16, name="h1_init", tag="h0")
            nc.vector.memset(h1, 0.0)
            h2 = gs.tile([H, NL], BF16, name="h2_init", tag="h0")
            nc.vector.memset(h2, 0.0)

            for t in range(T):
                # layer 0: gi precomputed in SBUF, injected via identity mm
                prz = psum(f"prza{t}", [128, NL])
                nc.tensor.matmul(prz, lhsT=id_bf, rhs=gi0rz[:, t, :],
                                 start=True, stop=False)
                nc.tensor.matmul(prz, lhsT=s_whh[0][:, 0:128], rhs=h1,
                                 start=False, stop=True)
                pu = psum(f"pua{t}", [H, NL])
                nc.tensor.matmul(pu, lhsT=s_bhnb0r, rhs=ones_row,
                                 start=True, stop=False)
                nc.tensor.matmul(pu, lhsT=s_whh[0][:, 128:192], rhs=h1,
                                 start=False, stop=True)
                h1 = gate_tail(0, t, h1, prz, pu, gi0n[:, t, :])

                # layer 1: input gates + biases accumulated straight in PSUM
                prz2 = psum(f"przb{t}", [128, NL])
                nc.tensor.matmul(prz2, lhsT=s_rzb1r, rhs=ones_row,
                                 start=True, stop=False)
                nc.tensor.matmul(prz2, lhsT=s_wih[1][:, 0:128], rhs=h1,
                                 start=False, stop=False)
                nc.tensor.matmul(prz2, lhsT=s_whh[1][:, 0:128], rhs=h2,
                                 start=False, stop=True)
                pb2 = psum(f"pgb{t}", [H, NL])
                nc.tensor.matmul(pb2, lhsT=s_ginb1r, rhs=ones_row,
                                 start=True, stop=False)
                nc.tensor.matmul(pb2, lhsT=s_wih[1][:, 128:192], rhs=h1,
                                 start=False, stop=True)
                pu2 = psum(f"pub{t}", [H, NL])
                nc.tensor.matmul(pu2, lhsT=s_bhnb1r, rhs=ones_row,
                                 start=True, stop=False)
                nc.tensor.matmul(pu2, lhsT=s_whh[1][:, 128:192], rhs=h2,
                                 start=False, stop=True)
                h2 = gate_tail(1, t, h2, prz2, pu2, pb2)

            # ---- head: BN+ReLU, lin_out, log_softmax
            hl = pb.tile([H + 1, NL], BF16, name="hl", tag="hl")
            nc.scalar.activation(hl[0:64, :], h2, AF.Relu,
                                 bias=s_bout, scale=s_scout)
            nc.vector.memset(hl[64:65, :], 1.0)
            plg = psum("plg", [C, NL])
            nc.tensor.matmul(plg, lhsT=s_loutw, rhs=hl, start=True, stop=True)
            lg = pb.tile([C, NL], F32, name="lg", tag="lg")
            nc.vector.tensor_copy(lg, plg)
            for c2 in range(2):
                cn = 128 if c2 == 0 else NL - 128
                pt = psum(f"pt{c2}", [128, C])
                nc.tensor.transpose(pt[0:cn, :],
                                    lg[:, c2 * 128:c2 * 128 + cn],
                                    id_f32[0:C, 0:C])
                mx = pb.tile([128, 1], F32, name=f"mx{c2}", tag="mx", bufs=2)
                nc.vector.reduce_max(out=mx[0:cn, :], in_=pt[0:cn, :],
                                     axis=mybir.AxisListType.X, negate=True)
                ex = pb.tile([128, C], F32, name=f"ex{c2}", tag="ex", bufs=2)
                nc.scalar.activation(ex[0:cn, :], pt[0:cn, :], AF.Exp,
                                     bias=mx[0:cn, :])
                sm = pb.tile([128, 1], F32, name=f"sm{c2}", tag="sm", bufs=2)
                nc.vector.reduce_sum(out=sm[0:cn, :], in_=ex[0:cn, :],
                                     axis=mybir.AxisListType.X)
                ln = pb.tile([128, 1], F32, name=f"ln{c2}", tag="ln", bufs=2)
                nc.scalar.activation(ln[0:cn, :], sm[0:cn, :], AF.Ln)
                b2 = pb.tile([128, 1], F32, name=f"b2{c2}", tag="b2", bufs=2)
                nc.vector.tensor_sub(b2[0:cn, :], mx[0:cn, :], ln[0:cn, :])
                osb = pb.tile([128, C], F32, name=f"osb{c2}", tag="osb",
                              bufs=2)
                nc.scalar.activation(osb[0:cn, :], pt[0:cn, :], AF.Identity,
                                     bias=b2[0:cn, :])
                nc.sync.dma_start(out=out[c2 * 128:c2 * 128 + cn, :],
                                  in_=osb[0:cn, :])


def _build():
    if "nc" in _CACHE:
        return _CACHE["nc"]
    nc = bacc.Bacc("TRN2", target_bir_lowering=False, debug=False,
                   enable_asserts=False, num_devices=W)
    with tile.TileContext(nc) as tc:
        _emit(nc, tc)
    nc.compile()
    _CACHE["nc"] = nc
    return nc


def kernel_run(inputs, trace=False):
    nc = _build()
    in_maps = _prep(inputs)
    res = run_bass_kernel_spmd(nc, in_maps, core_ids=list(range(W)),
                               trace=trace)
    outs = np.concatenate(
        [np.asarray(res.results[c]["out"], np.float32) for c in range(W)],
        axis=0)
    return outs, res


def kernel(**inputs) -> np.ndarray:
    outs, _ = kernel_run(inputs, trace=False)
    return outs


# revision 23
# speedup vs baseline: 1.2511x; 1.0053x over previous
"""Trainium2 Bass kernel: NeuralGrangerCausality (GCN + causal attention + GRU).

Strategy (8 NeuronCores, SPMD):
  - Phase A (T-sharded, 8 timesteps/core): lin_in -> causal softmax-matmul
    -> fusion -> GCN x2 (dense normalized-adjacency matmul, built on host)
    all as bf16 PE matmuls with fp32 PSUM accumulation.
    Per-timestep node tensors are kept feature-major [H, N]; aggregation
    matmuls contract over nodes with 2-timestep-stacked lhsT ([j, 2t*h]) for
    full 128-wide PE utilization.
  - Reshard T->N via AllToAll (bf16, 2MB).
  - Phase B (N-sharded, 250 nodes/core): 2-layer GRU recurrence (input-gate
    matmuls batched, per-step Whh matmuls + gate math), BN+ReLU, lin_out,
    log_softmax.

kernel(**inputs) takes the FULL inputs, preps/shards on host (numpy only:
index->dense adjacency, transposes, BN folding), runs the NEFF on cores 0-7
via run_bass_kernel_spmd, and concatenates the per-core [250, 10] outputs.
"""

import os
import sys

import numpy as np

for _p in ("/opt/trn_rl_repo", "/root/.axon_site/_ro/trn_rl_repo"):
    if os.path.isdir(_p) and _p not in sys.path:
        sys.path.append(_p)

import ml_dtypes  # noqa: E402

import concourse.bass as bass  # noqa: E402,F401
import concourse.mybir as mybir  # noqa: E402
import concourse.tile as tile  # noqa: E402
from concourse import bacc  # noqa: E402
from concourse.bass_utils import run_bass_kernel_spmd  # noqa: E402
from concourse.masks import make_identity  # noqa: E402

AF = mybir.ActivationFunctionType
F32 = mybir.dt.float32
BF16 = mybir.dt.bfloat16
BF = ml_dtypes.bfloat16

T, N, F_IN, H, C = 64, 2000, 32, 64, 10
W = 8            # cores
TL = T // W      # timesteps per core (phase A)
NL = N // W      # nodes per core (phase B)
EPS = 1e-5
NJ = (N + 127) // 128                       # node contraction chunks
JCH = [(j * 128, min(128, N - j * 128)) for j in range(NJ)]
NI = 4
IC = N // NI                                # 500-wide free-dim chunks
ICH = [(i * IC, IC) for i in range(NI)]

_CACHE: dict = {}


# --------------------------------------------------------------------------
# host-side prep
# --------------------------------------------------------------------------

def _prep(inputs):
    f32 = np.float32
    g = {k: np.asarray(v) for k, v in inputs.items()}

    x_seq = g["x_seq"].astype(f32)                       # [T, N, F]
    src = g["edge_index"][0].astype(np.int64)
    dst = g["edge_index"][1].astype(np.int64)
    ew = g["edge_weight"].astype(f32)

    # GCN normalization with self loops (PyG gcn_norm, fill 1)
    loops = np.arange(N, dtype=np.int64)
    src_f = np.concatenate([src, loops])
    dst_f = np.concatenate([dst, loops])
    w_f = np.concatenate([ew, np.ones(N, f32)])
    deg = np.zeros(N, f32)
    np.add.at(deg, dst_f, w_f)
    dis = (1.0 / np.sqrt(np.maximum(deg, 1e-12))).astype(f32)
    norm = dis[src_f] * w_f * dis[dst_f]
    at = np.zeros((N, N), f32)                           # at[j, i] = A[i, j]
    np.add.at(at, (src_f, dst_f), norm)

    # x_seq feature-major with an appended ones-row (bias via matmul aug)
    xs = np.concatenate(
        [x_seq.transpose(0, 2, 1), np.ones((T, 1, N), f32)], axis=1
    )                                                    # [T, F+1, N]

    cwt = np.ascontiguousarray(g["causal_weight"].astype(f32).T)  # [j, i]

    linw = np.concatenate(
        [g["lin_in_w"].astype(f32).T, g["lin_in_b"].astype(f32)[None]], axis=0
    )                                                    # [F+1, H]

    fw = g["fusion_w"].astype(f32)                       # [H, 2H]
    fusw1d = np.concatenate([fw[:, :H].T, fw[:, :H].T], 0)   # [2H, H]
    fusw2d = np.concatenate([fw[:, H:].T, fw[:, H:].T], 0)
    fusbd = np.tile(g["fusion_b"].astype(f32), 2)[:, None]   # [2H, 1]

    def gcn_fold(wk, bk, gk, bbk, mk, vk):
        sc = g[gk].astype(f32) / np.sqrt(g[vk].astype(f32) + EPS)
        wt = g[wk].astype(f32).T * sc[None, :]           # [H_in, H_out]
        bias = (g[bk].astype(f32) - g[mk].astype(f32)) * sc + g[bbk].astype(f32)
        return (np.concatenate([wt, wt], 0),
                np.tile(bias, 2)[:, None])

    w0d, b0d = gcn_fold("gcn_w0", "gcn_b0", "bn0_g", "bn0_b", "bn0_m", "bn0_v")
    w1d, b1d = gcn_fold("gcn_w1", "gcn_b1", "bn1_g", "bn1_b", "bn1_m", "bn1_v")

    p = {
        "cwt": cwt, "at": at.astype(BF),
        "linw": linw.astype(BF),
        "fusw1d": fusw1d.astype(BF), "fusw2d": fusw2d.astype(BF),
        "fusbd": fusbd,
        "w0d": w0d.astype(BF), "b0d": b0d,
        "w1d": w1d.astype(BF), "b1d": b1d,
    }
    for layer in (0, 1):
        wih = g[f"gru_wih{layer}"].astype(f32)            # [3H, H]
        whh = g[f"gru_whh{layer}"].astype(f32)
        bih = g[f"gru_bih{layer}"].astype(f32)
        bhh = g[f"gru_bhh{layer}"].astype(f32)
        p[f"wih{layer}"] = wih.T.astype(BF)               # [H, 3H]
        p[f"whh{layer}"] = whh.T.astype(BF)
    p["girzb0"] = (g["gru_bih0"].astype(f32)
                   + g["gru_bhh0"].astype(f32))[: 2 * H, None]
    p["ginb0"] = g["gru_bih0"].astype(f32)[2 * H:, None]
    # stacked-GRU: whh1 duplicated into partitions 64:128; per-bank bias
    # columns [L1-half; L2-half] (L1 biases already live in gi0)
    whh1t = g["gru_whh1"].astype(f32).T
    p["whh1d"] = np.concatenate([whh1t, whh1t], 0).astype(BF)   # [128, 3H]
    b1rz = (g["gru_bih1"].astype(f32) + g["gru_bhh1"].astype(f32))[: 2 * H]
    zz = np.zeros(H, f32)
    p["stk_rb"] = np.concatenate([zz, b1rz[:H]])[:, None]
    p["stk_zb"] = np.concatenate([zz, b1rz[H:]])[:, None]
    p["stk_zbn"] = -p["stk_zb"]
    p["stk_ub"] = np.concatenate([g["gru_bhh0"].astype(f32)[2 * H:],
                                  g["gru_bhh1"].astype(f32)[2 * H:]])[:, None]
    p["stk_bb"] = np.concatenate([zz,
                                  g["gru_bih1"].astype(f32)[2 * H:]])[:, None]

    scout = g["bnout_g"].astype(f32) / np.sqrt(g["bnout_v"].astype(f32) + EPS)
    p["scout"] = scout[:, None]
    p["bout"] = (g["bnout_b"].astype(f32)
                 - g["bnout_m"].astype(f32) * scout)[:, None]
    p["loutw"] = np.concatenate(
        [g["lin_out_w"].astype(f32).T, g["lin_out_b"].astype(f32)[None]], 0
    ).astype(BF)                                          # [H+1, C]

    xs_bf = xs.astype(BF)
    in_maps = []
    for c in range(W):
        m = dict(p)
        m["xs"] = np.ascontiguousarray(xs_bf[c * TL:(c + 1) * TL])
        in_maps.append(m)
    return in_maps


# --------------------------------------------------------------------------
# kernel IR
# --------------------------------------------------------------------------

def _emit(nc, tc):
    def param(name, shape, dt):
        return nc.dram_tensor(name, shape, dt, kind="ExternalInput").ap()

    xs = param("xs", [TL, F_IN + 1, N], BF16)
    cwt = param("cwt", [N, N], F32)
    at = param("at", [N, N], BF16)
    linw = param("linw", [F_IN + 1, H], BF16)
    fusw1d = param("fusw1d", [2 * H, H], BF16)
    fusw2d = param("fusw2d", [2 * H, H], BF16)
    fusbd = param("fusbd", [2 * H, 1], F32)
    w0d = param("w0d", [2 * H, H], BF16)
    b0d = param("b0d", [2 * H, 1], F32)
    w1d = param("w1d", [2 * H, H], BF16)
    b1d = param("b1d", [2 * H, 1], F32)
    wih = [param(f"wih{l}", [H, 3 * H], BF16) for l in (0, 1)]
    whh = [param(f"whh{l}", [H, 3 * H], BF16) for l in (0, 1)]
    girzb0 = param("girzb0", [2 * H, 1], F32)
    ginb0 = param("ginb0", [H, 1], F32)
    whh1d = param("whh1d", [2 * H, 3 * H], BF16)
    stk_rb = param("stk_rb", [2 * H, 1], F32)
    stk_zb = param("stk_zb", [2 * H, 1], F32)
    stk_zbn = param("stk_zbn", [2 * H, 1], F32)
    stk_ub = param("stk_ub", [2 * H, 1], F32)
    stk_bb = param("stk_bb", [2 * H, 1], F32)
    scout = param("scout", [H, 1], F32)
    bout = param("bout", [H, 1], F32)
    loutw = param("loutw", [H + 1, C], BF16)
    out = nc.dram_tensor("out", [NL, C], F32, kind="ExternalOutput").ap()

    with tc.tile_pool(name="consts", bufs=1) as cst, \
         tc.tile_pool(name="dram", bufs=1, space="DRAM") as dram, \
         tc.tile_pool(name="ps", bufs=8, space="PSUM") as ps:

        def psum(pn, pshape):
            return ps.tile(pshape, F32, tag="ps", name=pn,
                           padded_shape=[128, 512])

        # ---- constants into SBUF
        def cload(ap_, cn):
            t_ = cst.tile(list(ap_.shape), ap_.dtype, name=cn, tag=cn)
            nc.sync.dma_start(out=t_, in_=ap_)
            return t_

        s_linw = cload(linw, "s_linw")
        s_fusw1 = cload(fusw1d, "s_fusw1")
        s_fusw2 = cload(fusw2d, "s_fusw2")
        s_fusb = cload(fusbd, "s_fusb")
        s_w0 = cload(w0d, "s_w0")
        s_b0 = cload(b0d, "s_b0")
        s_w1 = cload(w1d, "s_w1")
        s_b1 = cload(b1d, "s_b1")
        s_wih = [cload(wih[l], f"s_wih{l}") for l in (0, 1)]
        s_whh = [cload(whh[l], f"s_whh{l}") for l in (0, 1)]
        s_girzb0 = cload(girzb0, "s_girzb0")
        s_ginb0 = cload(ginb0, "s_ginb0")
        s_whh1d = cload(whh1d, "s_whh1d")
        s_stk_rb = cload(stk_rb, "s_stk_rb")
        s_stk_zb = cload(stk_zb, "s_stk_zb")
        s_stk_zbn = cload(stk_zbn, "s_stk_zbn")
        s_stk_ub = cload(stk_ub, "s_stk_ub")
        s_stk_bb = cload(stk_bb, "s_stk_bb")
        s_scout = cload(scout, "s_scout")
        s_bout = cload(bout, "s_bout")
        s_loutw = cload(loutw, "s_loutw")

        ones_col = cst.tile([128, 1], BF16, name="ones_col", tag="ones_col")
        nc.vector.memset(ones_col, 1.0)
        ones_row = cst.tile([1, NL], BF16, name="ones_row", tag="ones_row")
        nc.vector.memset(ones_row, 1.0)
        id_bf = cst.tile([128, 128], BF16, name="id_bf", tag="id_bf")
        make_identity(nc, id_bf)
        id_f32 = cst.tile([16, 16], F32, name="id_f32", tag="id_f32")
        make_identity(nc, id_f32)

        a2a_in = [dram.tile([W, 2, H, NL], BF16, name=f"a2a_in{q}",
                            tag=f"a2a_in{q}") for q in range(TL // 2)]
        a2a_out = [dram.tile([W, 2, H, NL], BF16, name=f"a2a_out{q}",
                             tag=f"a2a_out{q}") for q in range(TL // 2)]

        # ================= PHASE A (T-sharded) =================
        with tc.tile_pool(name="pa", bufs=1) as pa, \
             tc.tile_pool(name="st", bufs=3) as st:

            # big per-t-pair stacked tiles
            x1nm = [pa.tile([128, NJ, 128], BF16, name=f"x1nm{q}",
                            tag=f"x1nm{q}") for q in range(TL // 2)]
            x1p = [pa.tile([128, N], BF16, name=f"x1p{q}", tag=f"x1p{q}")
                   for q in range(TL // 2)]
            xap = [pa.tile([128, N], BF16, name=f"xap{q}", tag=f"xap{q}")
                   for q in range(TL // 2)]
            x2p = [pa.tile([128, N], BF16, name=f"x2p{q}", tag=f"x2p{q}")
                   for q in range(TL // 2)]
            x3p = [pa.tile([128, N], BF16, name=f"x3p{q}", tag=f"x3p{q}")
                   for q in range(TL // 2)]
            x4p = [pa.tile([128, N], BF16, name=f"x4p{q}", tag=f"x4p{q}")
                   for q in range(TL // 2)]

            # ---- stage 1: x1 = relu(lin_in(x)) in both layouts
            for t in range(TL):
                q, o = t // 2, t % 2
                hs = slice(64 * o, 64 * o + 64)
                xst = st.tile([F_IN + 1, N], BF16, name=f"xs{t}", tag="xs")
                nc.sync.dma_start(out=xst, in_=xs[t])
                # feature-major into x1p halves
                for i, (i0, iw) in enumerate(ICH):
                    pfm = psum(f"pfm{t}_{i}", [128, IC])
                    nc.tensor.matmul(pfm[hs, :], lhsT=s_linw,
                                     rhs=xst[:, i0:i0 + iw],
                                     start=True, stop=True)
                    nc.vector.tensor_scalar_max(x1p[q][hs, i0:i0 + iw],
                                                pfm[hs, :], 0.0)
                # node-major into x1nm column halves
                for j, (j0, pj) in enumerate(JCH):
                    pnm = psum(f"pnm{t}_{j}", [128, H])
                    nc.tensor.matmul(pnm[0:pj, :], lhsT=xst[:, j0:j0 + pj],
                                     rhs=s_linw, start=True, stop=True)
                    nc.vector.tensor_scalar_max(
                        x1nm[q][0:pj, j, 64 * o:64 * o + 64],
                        pnm[0:pj, :], 0.0)

            # ---- stage 2: x_agg = softmax(cw) @ x1  (+ column rescale)
            for i, (i0, iw) in enumerate(ICH):
                pcs = psum(f"pcs{i}", [1, IC])
                pagg = [psum(f"pagg{i}_{q}", [128, IC])
                        for q in range(TL // 2)]
                for j, (j0, pj) in enumerate(JCH):
                    cwf = st.tile([128, IC], F32, name=f"cwf{i}_{j}",
                                  tag="cwf")
                    nc.sync.dma_start(out=cwf[0:pj, :],
                                      in_=cwt[j0:j0 + pj, i0:i0 + iw])
                    wct = st.tile([128, IC], BF16, name=f"wct{i}_{j}",
                                  tag="wct")
                    nc.scalar.activation(wct[0:pj, :], cwf[0:pj, :], AF.Exp)
                    nc.tensor.matmul(pcs, lhsT=ones_col[0:pj, :],
                                     rhs=wct[0:pj, :],
                                     start=(j == 0), stop=(j == NJ - 1))
                    for q in range(TL // 2):
                        nc.tensor.matmul(pagg[q],
                                         lhsT=x1nm[q][0:pj, j, :],
                                         rhs=wct[0:pj, :],
                                         start=(j == 0), stop=(j == NJ - 1))
                rinv = st.tile([1, IC], F32, name=f"rinv{i}", tag="rinv",
                               bufs=2)
                nc.vector.reciprocal(rinv, pcs)
                rb = st.tile([128, IC], F32, name=f"rb{i}", tag="rb", bufs=2)
                nc.gpsimd.partition_broadcast(rb, rinv)
                for q in range(TL // 2):
                    for o in (0, 1):
                        hs = slice(64 * o, 64 * o + 64)
                        nc.vector.tensor_mul(xap[q][hs, i0:i0 + iw],
                                             pagg[q][hs, :], rb[hs, :])

            # ---- stage 3: fusion x2 = relu(W1@x1 + W2@xagg + b)
            for q in range(TL // 2):
                for i, (i0, iw) in enumerate(ICH):
                    pf = psum(f"pf{q}_{i}", [128, IC])
                    for o in (0, 1):
                        hs = slice(64 * o, 64 * o + 64)
                        nc.tensor.matmul(pf[hs, :], lhsT=s_fusw1[hs, :],
                                         rhs=x1p[q][hs, i0:i0 + iw],
                                         start=True, stop=False)
                        nc.tensor.matmul(pf[hs, :], lhsT=s_fusw2[hs, :],
                                         rhs=xap[q][hs, i0:i0 + iw],
                                         start=False, stop=True)
                        nc.scalar.activation(x2p[q][hs, i0:i0 + iw],
                                             pf[hs, :], AF.Relu,
                                             bias=s_fusb[hs, :])

            # ---- GCN layer: z = (x @ Wsc) node-major, agg = A @ z, BN+ReLU
            def gcn_layer(xin, w_dup, b_dup, xout, evac):
                znm = [pa.tile([128, NJ, 128], BF16, name=f"znm{q}",
                               tag=f"znm{q}", bufs=2)
                       for q in range(TL // 2)]
                for t in range(TL):
                    q, o = t // 2, t % 2
                    hs = slice(64 * o, 64 * o + 64)
                    for j, (j0, pj) in enumerate(JCH):
                        pz = psum(f"pz{t}_{j}", [128, H])
                        nc.tensor.matmul(pz[0:pj, :],
                                         lhsT=xin[q][hs, j0:j0 + pj],
                                         rhs=w_dup[hs, :],
                                         start=True, stop=True)
                        nc.vector.tensor_copy(
                            znm[q][0:pj, j, 64 * o:64 * o + 64], pz[0:pj, :])
                for i, (i0, iw) in enumerate(ICH):
                    pagg = [psum(f"pag{i}_{q}", [128, IC])
                            for q in range(TL // 2)]
                    for j, (j0, pj) in enumerate(JCH):
                        atb = st.tile([128, IC], BF16, name=f"atb{i}_{j}",
                                      tag="atb", bufs=4)
                        nc.sync.dma_start(out=atb[0:pj, :],
                                          in_=at[j0:j0 + pj, i0:i0 + iw])
                        for q in range(TL // 2):
                            nc.tensor.matmul(pagg[q],
                                             lhsT=znm[q][0:pj, j, :],
                                             rhs=atb[0:pj, :],
                                             start=(j == 0),
                                             stop=(j == NJ - 1))
                    for q in range(TL // 2):
                        for o in (0, 1):
                            hs = slice(64 * o, 64 * o + 64)
                            evac(pagg[q], q, o, hs, i0, iw, b_dup, xout)

            def evac_bn(pagg, q, o, hs, i0, iw, b_dup, xout):
                nc.scalar.activation(xout[q][hs, i0:i0 + iw], pagg[hs, :],
                                     AF.Relu, bias=b_dup[hs, :])

            gcn_layer(x2p, s_w0, s_b0, x3p, evac_bn)
            gcn_layer(x3p, s_w1, s_b1, x4p, evac_bn)

            # ---- ship x4 to the all-to-all buffers (t-pair per DMA),
            # one collective per t-pair so reshard overlaps the tail
            for q in range(TL // 2):
                for d in range(W):
                    nc.sync.dma_start(
                        out=a2a_in[q][d],
                        in_=x4p[q][:, d * NL:(d + 1) * NL])
                nc.gpsimd.collective_compute(
                    "AllToAll", mybir.AluOpType.bypass,
                    replica_groups=[list(range(W))],
                    ins=[a2a_in[q].opt()], outs=[a2a_out[q].opt()])

        # ================= PHASE B (N-sharded GRU) =================
        with tc.tile_pool(name="pb", bufs=1) as pb, \
             tc.tile_pool(name="gs", bufs=4) as gs:

            x4all = pb.tile([H, W, TL, NL], BF16, name="x4all", tag="x4all")
            for q in range(TL // 2):
                for sct in range(W):
                    nc.sync.dma_start(
                        out=x4all[:, sct, 2 * q:2 * q + 2, :],
                        in_=a2a_out[q][sct].rearrange("t h n -> h t n"))
            x4f = x4all.rearrange("h s t n -> h (s t n)")

            # layer-0 input gates, batched over pairs of timesteps;
            # r and z land in separate base-0 tiles (uniform matmul
            # tile_position in the recurrence groups)
            gi0r = pb.tile([H, T, NL], BF16, name="gi0r", tag="gi0r")
            gi0z = pb.tile([H, T, NL], BF16, name="gi0z", tag="gi0z")
            gi0n = pb.tile([H, T, NL], BF16, name="gi0n", tag="gi0n")
            gi0rf = gi0r.rearrange("p t n -> p (t n)")
            gi0zf = gi0z.rearrange("p t n -> p (t n)")
            gi0nf = gi0n.rearrange("p t n -> p (t n)")
            for p2 in range(T // 2):
                csl = slice(2 * p2 * NL, (2 * p2 + 2) * NL)
                rhs = x4f[:, csl]
                prz = psum(f"prz{p2}", [128, 2 * NL])
                nc.tensor.matmul(prz, lhsT=s_wih[0][:, 0:128], rhs=rhs,
                                 start=True, stop=True)
                nc.scalar.activation(gi0rf[:, csl], prz[0:64, :],
                                     AF.Identity, bias=s_girzb0[0:64, :])
                nc.scalar.activation(gi0zf[:, csl], prz[64:128, :],
                                     AF.Identity, bias=s_girzb0[64:128, :])
                pn = psum(f"pn{p2}", [H, 2 * NL])
                nc.tensor.matmul(pn, lhsT=s_wih[0][:, 128:192], rhs=rhs,
                                 start=True, stop=True)
                nc.scalar.activation(gi0nf[:, csl], pn,
                                     AF.Identity, bias=s_ginb0)

            # Stacked 2-layer recurrence: wave k runs layer-0 step k in
            # partitions 0:64 and layer-1 step k-1 in partitions 64:128.
            # hs(k) = [h1(k); h2(k-1)]; every gate op is one 128-wide inst.
            id64 = id_bf[0:64, 0:64]

            def stk(nm, k, shape=None):
                return gs.tile(shape or [2 * H, NL], BF16,
                               name=f"{nm}{k}", tag=nm)

            hs = gs.tile([2 * H, NL], BF16, name="hs_init", tag="hs")
            nc.vector.memset(hs, 0.0)
            h2lo = gs.tile([H, NL], BF16, name="h2lo_init", tag="h2lo")
            nc.vector.memset(h2lo, 0.0)
            for k in range(T + 1):
                l1 = k < T          # layer-0 half active
                l2 = k > 0          # layer-1 half active
                pr = psum(f"pR{k}", [2 * H, NL])
                pz = psum(f"pZ{k}", [2 * H, NL])
                pu = psum(f"pU{k}", [2 * H, NL])
                pbk = psum(f"pB{k}", [2 * H, NL]) if l2 else None

                rmm, zmm, umm, bmm = [], [], [], []
                if l1:
                    rmm.append([(pr[0:64, :], id64, gi0r[:, k, :]),
                                (pr[0:64, :], s_whh[0][:, 0:64],
                                 hs[0:64, :])])
                    zmm.append([(pz[0:64, :], id64, gi0z[:, k, :]),
                                (pz[0:64, :], s_whh[0][:, 64:128],
                                 hs[0:64, :])])
                    umm.append([(pu[0:64, :], s_whh[0][:, 128:192],
                                 hs[0:64, :])])
                if l2:
                    rmm.append([(pr[64:128, :], s_wih[1][:, 0:64],
                                 hs[0:64, :]),
                                (pr[64:128, :], s_whh[1][:, 0:64], h2lo)])
                    zmm.append([(pz[64:128, :], s_wih[1][:, 64:128],
                                 hs[0:64, :]),
                                (pz[64:128, :], s_whh[1][:, 64:128], h2lo)])
                    umm.append([(pu[64:128, :], s_whh[1][:, 128:192],
                                 h2lo)])
                    bmm.append([(pbk[64:128, :], s_wih[1][:, 128:192],
                                 hs[0:64, :])])
                for bank in (rmm, zmm, umm, bmm):
                    for grp in bank:
                        for ix, (o_, l_, r_) in enumerate(grp):
                            nc.tensor.matmul(o_, lhsT=l_, rhs=r_,
                                             start=(ix == 0),
                                             stop=(ix == len(grp) - 1))
                lo, hi = (0 if l1 else 64), (128 if l2 else 64)
                sl = slice(lo, hi)
                r_t = stk("r", k)
                nc.scalar.activation(r_t[sl, :], pr[sl, :], AF.Sigmoid,
                                     bias=s_stk_rb[sl, :])
                z_t = stk("z", k)
                nc.scalar.activation(z_t[sl, :], pz[sl, :], AF.Sigmoid,
                                     bias=s_stk_zb[sl, :])
                zp_t = stk("zp", k)
                nc.scalar.activation(zp_t[sl, :], pz[sl, :], AF.Sigmoid,
                                     bias=s_stk_zbn[sl, :], scale=-1.0)
                zh_t = stk("zh", k)
                nc.vector.tensor_mul(zh_t[sl, :], z_t[sl, :], hs[sl, :])
                v_t = stk("v", k)
                nc.vector.scalar_tensor_tensor(
                    v_t[sl, :], pu[sl, :], s_stk_ub[sl, :], r_t[sl, :],
                    op0=mybir.AluOpType.add, op1=mybir.AluOpType.mult)
                w_t = stk("w", k)
                if l1:
                    nc.vector.tensor_add(w_t[0:64, :], v_t[0:64, :],
                                         gi0n[:, k, :])
                if l2:
                    nc.vector.scalar_tensor_tensor(
                        w_t[64:128, :], pbk[64:128, :],
                        s_stk_bb[64:128, :], v_t[64:128, :],
                        op0=mybir.AluOpType.add, op1=mybir.AluOpType.add)
                n_t = stk("n", k)
                nc.scalar.activation(n_t[sl, :], w_t[sl, :], AF.Tanh)
                m_t = stk("m", k)
                nc.vector.tensor_mul(m_t[sl, :], n_t[sl, :], zp_t[sl, :])
                hs_new = stk("hs", k)
                if not l2:
                    nc.vector.memset(hs_new[64:128, :], 0.0)
                nc.vector.tensor_add(hs_new[sl, :], m_t[sl, :], zh_t[sl, :])
                hs = hs_new
                if l2 and k < T:
                    h2lo = gs.tile([H, NL], BF16, name=f"h2lo{k}",
                                   tag="h2lo")
                    nc.vector.tensor_copy(h2lo, hs_new[64:128, :])
            h2 = hs[64:128, :]

            # ---- head: BN+ReLU, lin_out, log_softmax
            hl = pb.tile([H + 1, NL], BF16, name="hl", tag="hl")
            nc.scalar.activation(hl[0:64, :], h2, AF.Relu,
                                 bias=s_bout, scale=s_scout)
            nc.vector.memset(hl[64:65, :], 1.0)
            plg = psum("plg", [C, NL])
            nc.tensor.matmul(plg, lhsT=s_loutw, rhs=hl, start=True, stop=True)
            lg = pb.tile([C, NL], F32, name="lg", tag="lg")
            nc.vector.tensor_copy(lg, plg)
            for c2 in range(2):
                cn = 128 if c2 == 0 else NL - 128
                pt = psum(f"pt{c2}", [128, C])
                nc.tensor.transpose(pt[0:cn, :],
                                    lg[:, c2 * 128:c2 * 128 + cn],
                                    id_f32[0:C, 0:C])
                mx = pb.tile([128, 1], F32, name=f"mx{c2}", tag="mx", bufs=2)
                nc.vector.reduce_max(out=mx[0:cn, :], in_=pt[0:cn, :],
                                     axis=mybir.AxisListType.X, negate=True)
                ex = pb.tile([128, C], F32, name=f"ex{c2}", tag="ex", bufs=2)
                nc.scalar.activation(ex[0:cn, :], pt[0:cn, :], AF.Exp,
                                     bias=mx[0:cn, :])
                sm = pb.tile([128, 1], F32, name=f"sm{c2}", tag="sm", bufs=2)
                nc.vector.reduce_sum(out=sm[0:cn, :], in_=ex[0:cn, :],
                                     axis=mybir.AxisListType.X)
                ln = pb.tile([128, 1], F32, name=f"ln{c2}", tag="ln", bufs=2)
                nc.scalar.activation(ln[0:cn, :], sm[0:cn, :], AF.Ln)
                b2 = pb.tile([128, 1], F32, name=f"b2{c2}", tag="b2", bufs=2)
                nc.vector.tensor_sub(b2[0:cn, :], mx[0:cn, :], ln[0:cn, :])
                osb = pb.tile([128, C], F32, name=f"osb{c2}", tag="osb",
                              bufs=2)
                nc.scalar.activation(osb[0:cn, :], pt[0:cn, :], AF.Identity,
                                     bias=b2[0:cn, :])
                nc.sync.dma_start(out=out[c2 * 128:c2 * 128 + cn, :],
                                  in_=osb[0:cn, :])


def _build():
    if "nc" in _CACHE:
        return _CACHE["nc"]
    nc = bacc.Bacc("TRN2", target_bir_lowering=False, debug=False,
                   enable_asserts=False, num_devices=W)
    with tile.TileContext(nc) as tc:
        _emit(nc, tc)
    nc.compile()
    _CACHE["nc"] = nc
    return nc


def kernel_run(inputs, trace=False):
    nc = _build()
    in_maps = _prep(inputs)
    res = run_bass_kernel_spmd(nc, in_maps, core_ids=list(range(W)),
                               trace=trace)
    outs = np.concatenate(
        [np.asarray(res.results[c]["out"], np.float32) for c in range(W)],
        axis=0)
    return outs, res


def kernel(**inputs) -> np.ndarray:
    outs, _ = kernel_run(inputs, trace=False)
    return outs
